# revision 1
# baseline (speedup 1.0000x reference)
"""Trainium2 Bass kernel for nn_DiffTime (embedding_lookup, 8 NeuronCores).

Computation (see reference):
    h1 = tanh(times * h1_k + h1_b)            [B, 100]
    tv = tanh(h1 @ h2_k + h2_b)               [B, 100]
    mat_x = (emb_x @ evoke_k + evoke_b)       [B, 100p, 100h]   (x in {target, context})
    mv_x = einsum('bph,bh->bp', mat_x, tv)    [B, 100]
    vect_x = mv_x @ last_k + last_b           [B, 300]
    logits = sum(vect_t * vect_c, -1)         [B]
    out = mean(softplus(logits) - logits * labels)

Strategy (data-parallel over batch, 2048 items/core, no collectives):

* Embedding rows are gathered on-device with dma_gather (int16 indices:
  the 100k vocab is split into 4 sorted segments of <=32768 rows, gathered
  to a DRAM scratch, then a second gather restores batch order).  Gathers
  run in row-major (non-transpose) mode -- one contiguous descriptor per
  row; the [e, b] lhsT layout is recovered with PE transposes per chunk.

* tv[b,:] is a function of the single scalar times[b], so its rows live on
  a smooth 1-D curve in R^100.  The curve's SVD (host precompute from the
  MLP weights only -- input independent) collapses: rank 16 reproduces tv
  to ~3e-11.  The kernel therefore contracts emb with
  Wr[e,(p,k)] = sum_h evoke[e,p*100+h]*Vr[h,k]  (k = 16 basis coeffs)
  and forms mv[b,p] = sum_k matU[b,p,k] * c[b,k], c = tv @ Vr -- an
  r=16 contraction instead of 100, cutting TensorE+VectorE work ~6x.
  Validated end-to-end (bf16 pipeline): final-scalar rel err 1.2e-6.

* logits use the Gram matrix Gh = last_kh @ last_kh.T computed on device
  (homogeneous coordinate folds last_b).  Per-sample losses are computed
  batched [128, 16] at the end (softplus via Relu + Ln(1+Exp(-|x|))); each
  core returns a partial sum and the host adds 8 scalars.
"""

import sys

for _p in ("/opt/trn_rl_repo", "/opt/trn_rl_repo/concourse"):
    if _p not in sys.path:
        sys.path.insert(0, _p)

from contextlib import ExitStack

import ml_dtypes
import numpy as np

import concourse.bacc as bacc
import concourse.bass as bass
import concourse.tile as tile
from concourse import mybir
from concourse.bass_utils import run_bass_kernel_spmd

F32 = mybir.dt.float32
BF16 = mybir.dt.bfloat16
I16 = mybir.dt.int16
AF = mybir.ActivationFunctionType
AX = mybir.AxisListType
OP = mybir.AluOpType

N_CORES = 8
B = 16384
BC = B // N_CORES          # 2048 batch items per core
NB = BC // 128             # 16 chunks of 128 batch rows
V = 100000
EMB = 300
EPAD = 384                 # padded embedding row (col 300 = 1.0, rest 0)
H = 100                    # h1 = h2 = prod dims
R = 16                     # tv-curve basis rank
NPR = H * R                # 1600 contracted columns
CH = 25 * R                # 400: matmul moving chunk (25 p-groups)
NCH = NPR // CH            # 4 chunks
PG = CH // R               # 25 p's per chunk
MH = H + 1                 # homogeneous mv size
SEG_BASE = [0, 32768, 65536, 98304]
SEG_CAP = [768, 768, 768, 128]   # fixed (SPMD-stable) per-segment capacity
S_TOT = sum(SEG_CAP)             # 2432 scratch rows
assert S_TOT % 128 == 0


def _wrap16(v):
    """int16 index array -> dma_gather SBUF layout [128, len//16]."""
    v = np.asarray(v, dtype=np.int16)
    a = v.reshape(-1, 16).T          # [16, len/16]; slot j at [j%16, j//16]
    return np.tile(a, (8, 1))        # replicate across the 8 q7 cores


def _prep_indices(idx):
    """Sort a core's indices into int16-addressable segments.

    Returns (seg_idx [128, S_TOT//16], realign [128, BC//16]) int16 arrays.
    seg_idx holds per-segment local indices (padded with 0); realign maps
    original batch position j -> scratch row of its gathered embedding.
    """
    idx = np.asarray(idx).astype(np.int64)
    assert idx.shape == (BC,)
    order = np.argsort(idx, kind="stable")
    sidx = idx[order]
    bounds = np.searchsorted(sidx, SEG_BASE + [V])
    seg_cols = []
    scratch_rows = np.empty(BC, dtype=np.int64)
    off = 0
    for s in range(4):
        lo, hi = bounds[s], bounds[s + 1]
        n = hi - lo
        assert n <= SEG_CAP[s], f"segment {s} overflow: {n} > {SEG_CAP[s]}"
        local = np.zeros(SEG_CAP[s], dtype=np.int16)
        local[:n] = sidx[lo:hi] - SEG_BASE[s]
        seg_cols.append(_wrap16(local))
        scratch_rows[lo:hi] = off + np.arange(n)
        off += SEG_CAP[s]
    realign = np.empty(BC, dtype=np.int64)
    realign[order] = scratch_rows
    return np.hstack(seg_cols), _wrap16(realign)


def _build_kernel(ctx: ExitStack, tc: "tile.TileContext", io: dict):
    nc = tc.nc

    cpool = ctx.enter_context(tc.tile_pool(name="const", bufs=1))
    gpool = ctx.enter_context(tc.tile_pool(name="gather", bufs=2))
    dpool = ctx.enter_context(tc.tile_pool(name="scratch", bufs=1, space="DRAM"))
    pmm = ctx.enter_context(tc.tile_pool(name="pmm", bufs=4, space="PSUM"))
    pmisc = ctx.enter_context(tc.tile_pool(name="pmisc", bufs=3, space="PSUM"))
    wpool = ctx.enter_context(tc.tile_pool(name="work", bufs=4))
    tvpool = ctx.enter_context(tc.tile_pool(name="tvp", bufs=3))
    lpool = ctx.enter_context(tc.tile_pool(name="loss", bufs=2))

    # ---- resident constants --------------------------------------------
    wr = [cpool.tile([128, NPR], BF16, tag=f"wr{j}", name=f"wr{j}")
          for j in range(3)]
    for j in range(3):
        nc.sync.dma_start(out=wr[j][:], in_=io["wr"][128 * j:128 * (j + 1), :])
    h2kb = cpool.tile([H + 1, H], F32, tag="h2kb")
    nc.sync.dma_start(out=h2kb[:], in_=io["h2kb"][:, :])
    h1k = cpool.tile([H, 1], F32, tag="h1k")
    nc.sync.dma_start(out=h1k[:], in_=io["h1k"][:, :])
    h1b = cpool.tile([H, 1], F32, tag="h1b")
    nc.sync.dma_start(out=h1b[:], in_=io["h1b"][:, :])
    vr = cpool.tile([H, R], F32, tag="vr")
    nc.sync.dma_start(out=vr[:], in_=io["vr"][:, :])
    lastkh = cpool.tile([MH, EMB], F32, tag="lastkh")
    nc.sync.dma_start(out=lastkh[:], in_=io["lastkh"][:, :])
    ident = cpool.tile([128, 128], F32, tag="ident")
    nc.sync.dma_start(out=ident[:], in_=io["ident"][:, :])
    identb = cpool.tile([128, 128], BF16, tag="identb")
    nc.sync.dma_start(out=identb[:], in_=io["identb"][:, :])
    times = cpool.tile([1, BC], F32, tag="times")
    nc.sync.dma_start(out=times[:], in_=io["times"][:, :])
    labels = cpool.tile([128, NB], F32, tag="labels")
    nc.sync.dma_start(out=labels[:], in_=io["labels"][:, :])
    idx_sb = {}
    for br in ("t", "c"):
        idx_sb[br] = cpool.tile([128, S_TOT // 16], I16, tag=f"idx_{br}",
                                name=f"idx_{br}")
        nc.sync.dma_start(out=idx_sb[br][:], in_=io[f"idx_{br}"][:, :])
        idx_sb[br + "r"] = cpool.tile([128, BC // 16], I16, tag=f"rel_{br}",
                                      name=f"rel_{br}")
        nc.sync.dma_start(out=idx_sb[br + "r"][:], in_=io[f"rel_{br}"][:, :])
    ones1 = cpool.tile([1, H], F32, tag="ones1")
    nc.vector.memset(ones1[:], 1.0)
    ones128 = cpool.tile([128, 1], F32, tag="ones128")
    nc.vector.memset(ones128[:], 1.0)

    # ---- gathers: table segments -> scratch -> batch-ordered rows ------
    # Row-major mode keeps one contiguous 768B descriptor per row.  Segment
    # gathers for both branches are issued before either realign so the
    # gpsimd queue stays busy while evict DMAs complete.
    scratch = {}
    for br, tab in (("t", io["ttab"]), ("c", io["ctab"])):
        scratch[br] = dpool.tile([S_TOT, EPAD], BF16, tag=f"scratch_{br}",
                                 name=f"scratch_{br}")
        off = 0
        for s in range(4):
            cap = SEG_CAP[s]
            seg_len = min(32768, V - SEG_BASE[s])
            g = gpool.tile([128, cap // 128, EPAD], BF16, tag="segg",
                           name=f"segg_{br}{s}")
            nc.gpsimd.dma_gather(
                g[:],
                tab[SEG_BASE[s]:SEG_BASE[s] + seg_len, :],
                idx_sb[br][:, off // 16:(off + cap) // 16],
                cap, cap, EPAD, queue_num=0,
            )
            sview = scratch[br][off:off + cap, :].rearrange(
                "(c p) e -> p c e", p=128)
            nc.scalar.dma_start(out=sview, in_=g[:])
            off += cap
    emb = {}
    for br in ("t", "c"):
        e = cpool.tile([128, NB, EPAD], BF16, tag=f"emb_{br}", name=f"emb_{br}")
        nc.gpsimd.dma_gather(
            e[:], scratch[br][:, :], idx_sb[br + "r"][:], BC, BC, EPAD,
            queue_num=0, single_packet=False,
        )
        emb[br] = e

    # ---- Gh = last_kh @ last_kh.T on device ----------------------------
    ech = [(0, 128), (128, 128), (256, EMB - 256)]
    lkT = []
    for j, (e0, en) in enumerate(ech):
        tp = pmisc.tile([en, MH], F32, tag="pm", name=f"ptrans{j}")
        nc.tensor.transpose(tp[:], lastkh[:, e0:e0 + en], ident[0:MH, 0:MH])
        t = cpool.tile([en, MH], F32, tag=f"lkT{j}", name=f"lkT{j}")
        nc.scalar.copy(t[:], tp[:])
        lkT.append(t)
    ghp = pmisc.tile([MH, MH], F32, tag="pm")
    for j in range(3):
        nc.tensor.matmul(ghp[:], lkT[j][:], lkT[j][:], start=(j == 0),
                         stop=(j == 2))
    gh = cpool.tile([MH, MH], F32, tag="gh")
    nc.scalar.copy(gh[:], ghp[:])

    # ---- time MLP -> tv basis coefficients c[b, :R] per chunk ----------
    c_all = []
    for c in range(NB):
        bcast = pmisc.tile([H, 128], F32, tag="pm", name="pbcast")
        nc.tensor.matmul(bcast[:], ones1[:],
                         times[0:1, 128 * c:128 * (c + 1)],
                         start=True, stop=True)
        h1T = tvpool.tile([H + 1, 128], F32, tag="h1T")
        nc.vector.memset(h1T[:], 1.0)
        nc.scalar.activation(h1T[0:H, :], bcast[:], AF.Tanh, bias=h1b[:],
                             scale=h1k[:])
        tvp = pmisc.tile([H, 128], F32, tag="pm", name="ptv")
        nc.tensor.matmul(tvp[:], h2kb[:], h1T[:], start=True, stop=True)
        tvT = tvpool.tile([H, 128], F32, tag="tvT")
        nc.scalar.activation(tvT[:], tvp[:], AF.Tanh)
        cfp = pmisc.tile([128, R], F32, tag="pm", name="pcf")
        nc.tensor.matmul(cfp[:], tvT[:], vr[:], start=True, stop=True)
        cb = tvpool.tile([128, R], BF16, tag="cb")
        nc.scalar.copy(cb[:], cfp[:])
        ctile = cpool.tile([128, CH], BF16, tag=f"ct_{c}", name=f"ct_{c}")
        nc.vector.tensor_copy(ctile[:, 0:R], cb[:])
        w = R
        while w < CH:
            n = min(w, CH - w)
            nc.vector.tensor_copy(ctile[:, w:w + n], ctile[:, 0:n])
            w += n
        c_all.append(ctile)

    # ---- main loop: matU = embT @ Wr ; mv = sum_k matU * c -------------
    def branch_mv(br, c):
        mv = cpool.tile([128, MH], F32, tag=f"mv_{br}{c}", name=f"mv_{br}{c}")
        nc.vector.memset(mv[:, H:MH], 1.0)
        lhs = []
        for j in range(3):
            tpp = pmisc.tile([128, 128], BF16, tag="pm", name=f"ptr{j}")
            nc.tensor.transpose(
                tpp[:], emb[br][:, c, 128 * j:128 * (j + 1)], identb[:])
            et = wpool.tile([128, 128], BF16, tag=f"embT{j}", name=f"embT{j}")
            nc.scalar.copy(et[:], tpp[:])
            lhs.append(et[:])
        mps = [pmm.tile([128, CH], F32, tag="mp", name=f"mp{n}")
               for n in range(NCH)]
        for j in range(3):
            for n in range(NCH):
                nc.tensor.matmul(
                    mps[n][:], lhs[j], wr[j][:, CH * n:CH * (n + 1)],
                    start=(j == 0), stop=(j == 2),
                )
        for n in range(NCH):
            ms = wpool.tile([128, CH], BF16, tag="ms")
            nc.scalar.copy(ms[:], mps[n][:])
            prod = wpool.tile([128, CH], BF16, tag="prod")
            nc.vector.tensor_mul(prod[:], ms[:], c_all[c][:])
            nc.vector.reduce_sum(
                out=mv[:, PG * n:PG * (n + 1)],
                in_=prod[:].rearrange("p (a k) -> p a k", k=R),
                axis=AX.X,
            )
        return mv

    mvt = [branch_mv("t", c) for c in range(NB)]

    logits = cpool.tile([128, NB], F32, tag="logits")
    for c in range(NB):
        mvc = branch_mv("c", c)
        tp = pmisc.tile([MH, 128], F32, tag="pm", name="pmvT")
        nc.tensor.transpose(tp[:], mvt[c][:], ident[:])
        mvtT = lpool.tile([MH, 128], F32, tag="mvtT")
        nc.scalar.copy(mvtT[:], tp[:])
        mg = pmisc.tile([128, MH], F32, tag="pm", name="pmg")
        nc.tensor.matmul(mg[:], mvtT[:], gh[:], start=True, stop=True)
        junk = lpool.tile([128, MH], F32, tag="ttrjunk")
        nc.vector.tensor_mul(junk[:], mg[:], mvc[:])
        nc.vector.reduce_sum(out=logits[:, c:c + 1], in_=junk[:], axis=AX.X)

    # ---- batched loss tail: softplus(l) - l*y over [128, NB] -----------
    ab = lpool.tile([128, NB], F32, tag="ab")
    nc.scalar.activation(ab[:], logits[:], AF.Abs)
    ex = lpool.tile([128, NB], F32, tag="ex")
    nc.scalar.activation(ex[:], ab[:], AF.Exp, scale=-1.0)
    l1p = lpool.tile([128, NB], F32, tag="l1p")
    nc.scalar.activation(l1p[:], ex[:], AF.Ln, bias=1.0)
    rl = lpool.tile([128, NB], F32, tag="rl")
    nc.scalar.activation(rl[:], logits[:], AF.Relu)
    sp = lpool.tile([128, NB], F32, tag="sp")
    nc.vector.tensor_add(sp[:], rl[:], l1p[:])
    ll = lpool.tile([128, NB], F32, tag="ll")
    nc.vector.tensor_mul(ll[:], logits[:], labels[:])
    dvec = lpool.tile([128, NB], F32, tag="dvec")
    nc.vector.tensor_sub(dvec[:], sp[:], ll[:])

    srow = cpool.tile([128, 1], F32, tag="srow")
    nc.vector.reduce_sum(out=srow[:], in_=dvec[:], axis=AX.X)
    fin = pmisc.tile([1, 1], F32, tag="pm", name="pfin")
    nc.tensor.matmul(fin[:], srow[:], ones128[:], start=True, stop=True)
    res = cpool.tile([1, 1], F32, tag="res")
    nc.scalar.copy(res[:], fin[:])
    nc.sync.dma_start(out=io["out"][:, :], in_=res[:])


_PROGRAM = None


def _get_program():
    global _PROGRAM
    if _PROGRAM is not None:
        return _PROGRAM
    nc = bacc.Bacc("TRN2", target_bir_lowering=False, debug=False,
                   num_devices=N_CORES)
    io = {
        "ttab": nc.dram_tensor("ttab", [V, EPAD], BF16, kind="ExternalInput").ap(),
        "ctab": nc.dram_tensor("ctab", [V, EPAD], BF16, kind="ExternalInput").ap(),
        "wr": nc.dram_tensor("wr", [EPAD, NPR], BF16, kind="ExternalInput").ap(),
        "vr": nc.dram_tensor("vr", [H, R], F32, kind="ExternalInput").ap(),
        "h2kb": nc.dram_tensor("h2kb", [H + 1, H], F32, kind="ExternalInput").ap(),
        "h1k": nc.dram_tensor("h1k", [H, 1], F32, kind="ExternalInput").ap(),
        "h1b": nc.dram_tensor("h1b", [H, 1], F32, kind="ExternalInput").ap(),
        "lastkh": nc.dram_tensor("lastkh", [MH, EMB], F32, kind="ExternalInput").ap(),
        "ident": nc.dram_tensor("ident", [128, 128], F32, kind="ExternalInput").ap(),
        "identb": nc.dram_tensor("identb", [128, 128], BF16, kind="ExternalInput").ap(),
        "times": nc.dram_tensor("times", [1, BC], F32, kind="ExternalInput").ap(),
        "labels": nc.dram_tensor("labels", [128, NB], F32, kind="ExternalInput").ap(),
        "idx_t": nc.dram_tensor("idx_t", [128, S_TOT // 16], I16, kind="ExternalInput").ap(),
        "idx_c": nc.dram_tensor("idx_c", [128, S_TOT // 16], I16, kind="ExternalInput").ap(),
        "rel_t": nc.dram_tensor("rel_t", [128, BC // 16], I16, kind="ExternalInput").ap(),
        "rel_c": nc.dram_tensor("rel_c", [128, BC // 16], I16, kind="ExternalInput").ap(),
        "out": nc.dram_tensor("out", [1, 1], F32, kind="ExternalOutput").ap(),
    }
    with tile.TileContext(nc) as tc:
        with ExitStack() as ctx:
            _build_kernel(ctx, tc, io)
    nc.compile()
    _PROGRAM = nc
    return nc


def _pad_table(tab):
    out = np.zeros((V, EPAD), dtype=ml_dtypes.bfloat16)
    out[:, :EMB] = np.asarray(tab).astype(ml_dtypes.bfloat16)
    out[:, EMB] = 1.0
    return out


def _tv_basis(h1_k, h1_b, h2_k, h2_b):
    """Top-R right singular basis of the tv curve (weights-only precompute)."""
    g = np.linspace(0.0, 1.0, 8193, dtype=np.float64).reshape(-1, 1)
    h1 = np.tanh(g @ np.asarray(h1_k, np.float64).reshape(1, H)
                 + np.asarray(h1_b, np.float64).reshape(H))
    tvg = np.tanh(h1 @ np.asarray(h2_k, np.float64)
                  + np.asarray(h2_b, np.float64).reshape(H))
    _, _, vt = np.linalg.svd(tvg, full_matrices=False)
    return np.ascontiguousarray(vt[:R].T)          # [100, R]


def build_in_maps(targets, contexts, times, labels, targetemb, contextemb,
                  h1_k, h1_b, h2_k, h2_b, evoke_k, evoke_b, last_k, last_b):
    ttab = _pad_table(targetemb)
    ctab = _pad_table(contextemb)
    vrb = _tv_basis(h1_k, h1_b, h2_k, h2_b)        # [100, R] float64
    evoke_pad = np.zeros((EPAD, H * H), dtype=np.float64)
    evoke_pad[:EMB, :] = np.asarray(evoke_k, np.float64)
    evoke_pad[EMB, :] = np.asarray(evoke_b, np.float64)
    # Wr[e, (p, k)] = sum_h evoke_pad[e, p*100+h] * Vr[h, k]
    wrm = (evoke_pad.reshape(EPAD * H, H) @ vrb).reshape(EPAD, NPR)
    wrm = wrm.astype(ml_dtypes.bfloat16)
    h2kb = np.vstack([np.asarray(h2_k), np.asarray(h2_b).reshape(1, H)]
                     ).astype(np.float32)
    h1kc = np.asarray(h1_k).reshape(1, H).T.astype(np.float32).copy()
    h1bc = np.asarray(h1_b).reshape(H, 1).astype(np.float32).copy()
    lastkh = np.vstack([np.asarray(last_k), np.asarray(last_b).reshape(1, EMB)]
                       ).astype(np.float32)
    ident = np.eye(128, dtype=np.float32)
    identb = np.eye(128, dtype=ml_dtypes.bfloat16)
    targets = np.asarray(targets)
    contexts = np.asarray(contexts)
    times = np.asarray(times).astype(np.float32)
    labels = np.asarray(labels).astype(np.float32)

    in_maps = []
    for k in range(N_CORES):
        sl = slice(k * BC, (k + 1) * BC)
        idx_t, rel_t = _prep_indices(targets[sl])
        idx_c, rel_c = _prep_indices(contexts[sl])
        in_maps.append({
            "ttab": ttab, "ctab": ctab, "wr": wrm,
            "vr": vrb.astype(np.float32), "h2kb": h2kb,
            "h1k": h1kc, "h1b": h1bc, "lastkh": lastkh, "ident": ident,
            "identb": identb,
            "times": times[sl].reshape(1, BC),
            "labels": labels[sl].reshape(NB, 128).T.copy(),
            "idx_t": idx_t, "idx_c": idx_c, "rel_t": rel_t, "rel_c": rel_c,
        })
    return in_maps


def kernel(**inputs) -> np.ndarray:
    nc = _get_program()
    in_maps = build_in_maps(**inputs)
    r = run_bass_kernel_spmd(nc, in_maps, list(range(N_CORES)))
    total = np.float64(0.0)
    for m in r.results:
        total += np.float64(m["out"][0, 0])
    return np.float32(total / B)



# revision 7
# speedup vs baseline: 2.7766x; 2.7766x over previous
"""Trainium2 Bass kernel for nn_DiffTime (embedding_lookup, 8 NeuronCores).

Reference computation:
    h1 = tanh(times * h1_k + h1_b)            [B, 100]
    tv = tanh(h1 @ h2_k + h2_b)               [B, 100]
    mat_x = (emb_x @ evoke_k + evoke_b)       [B, 100p, 100h]   (x in {target, context})
    mv_x = einsum('bph,bh->bp', mat_x, tv)    [B, 100]
    vect_x = mv_x @ last_k + last_b           [B, 300]
    logits = sum(vect_t * vect_c, -1)         [B]
    out = mean(softplus(logits) - logits * labels)

Kernel strategy (data-parallel, 2048 items/core, no collectives):

* tv rows lie on a smooth 1-D curve in R^100; an affine rank-4 basis
  (mean + 3 SVD directions of the centered curve, c0 == 1 by a
  homogeneous-coordinate trick) reproduces the final loss to ~4e-6.
  The h-contraction is folded into the weights on the host:
  Wr[e,(p,k)] = sum_h evoke_pad[e,p*100+h]*B_aff[h,k], so the kernel
  contracts emb (384-padded, homogeneous col 300 == 1) against a
  [384, 404] matrix and reduces over k=4 with a broadcast coefficient
  tile.  The Gram matrix Gh = lastkh @ lastkh.T (which turns the two
  [B,300] branch vectors into a [101]x[101] bilinear form) is folded
  into the context branch weights as well, so logits are a single
  fused multiply-reduce of the two [128,101] mv tiles.

* Gathers are single-stage on both branches (no scratch / realign):
  - batch items are assigned to cores by a global argsort of targets,
    so each core's target rows fall inside one 32768-row table window
    (span ~12.5k) => one 2048-row int16 dma_gather from a per-core
    window slice fed as input;
  - within each core, items are processed in context-sorted order
    (the loss is an order-invariant mean, so any processing order
    works as long as times/labels/indices are permuted consistently);
    the sorted contexts are cut at ranks 512/1024/1536 and gathered
    with four 512-row dma_gathers from per-core percentile windows
    (span of 512 sorted uniform draws ~26k < 32768).

* emb transposes ([b,e] -> [e,b] for the PE contraction) use the XBAR
  dma_start_transpose (SBUF->SBUF, [128,384] -> [128,3,128]) on the
  otherwise-idle SP queue instead of PE transposes + PSUM evictions.
"""

import sys

for _p in ("/opt/trn_rl_repo", "/opt/trn_rl_repo/concourse"):
    if _p not in sys.path:
        sys.path.insert(0, _p)

from contextlib import ExitStack

import ml_dtypes
import numpy as np

import concourse.bacc as bacc
import concourse.bass as bass
import concourse.tile as tile
from concourse import mybir
from concourse.bass_utils import run_bass_kernel_spmd

F32 = mybir.dt.float32
BF16 = mybir.dt.bfloat16
I16 = mybir.dt.int16
AF = mybir.ActivationFunctionType
AX = mybir.AxisListType
OP = mybir.AluOpType

N_CORES = 8
B = 16384
BC = B // N_CORES          # 2048 batch items per core
NB = BC // 128             # 16 chunks of 128 batch rows
V = 100000
EMB = 300
EPAD = 384                 # padded embedding row (col 300 = 1.0, rest 0)
H = 100
MH = H + 1                 # homogeneous mv size
R = 4                      # affine tv-basis rank (c0 == 1)
NPR = MH * R               # 404 contracted columns
W = 32768                  # per-core table window (int16-addressable)
SEG = 512                  # context gather piece (4 x 512 = 2048)

USE_DMA_TRANSPOSE = True

LAST_PERMS = None          # debug: per-core batch permutation of last build


def _wrap16(v):
    """int16 index array -> dma_gather SBUF layout [128, len//16]."""
    v = np.asarray(v, dtype=np.int16)
    a = v.reshape(-1, 16).T
    return np.tile(a, (8, 1))


def _build_kernel(ctx: ExitStack, tc: "tile.TileContext", io: dict):
    nc = tc.nc

    cpool = ctx.enter_context(tc.tile_pool(name="const", bufs=1))
    wpool = ctx.enter_context(tc.tile_pool(name="work", bufs=4))
    tvpool = ctx.enter_context(tc.tile_pool(name="tvp", bufs=3))
    lpool = ctx.enter_context(tc.tile_pool(name="loss", bufs=2))
    pmm = ctx.enter_context(tc.tile_pool(name="pmm", bufs=3, space="PSUM"))
    ptv = ctx.enter_context(tc.tile_pool(name="ptv", bufs=2, space="PSUM"))
    if not USE_DMA_TRANSPOSE:
        ptr = ctx.enter_context(tc.tile_pool(name="ptr", bufs=3, space="PSUM"))

    # ---- small resident constants (SP queue: these come first so the
    # gathers, which only need the index tiles, start immediately) ------
    idx_t = cpool.tile([128, BC // 16], I16, tag="idx_t")
    nc.sync.dma_start(out=idx_t[:], in_=io["idx_t"][:, :])
    idx_c = cpool.tile([128, BC // 16], I16, tag="idx_c")
    nc.sync.dma_start(out=idx_c[:], in_=io["idx_c"][:, :])
    times = cpool.tile([1, BC], BF16, tag="times")
    nc.sync.dma_start(out=times[:], in_=io["times"][:, :])
    h1k = cpool.tile([H, 1], F32, tag="h1k")
    nc.sync.dma_start(out=h1k[:], in_=io["h1k"][:, :])
    h1b = cpool.tile([H, 1], F32, tag="h1b")
    nc.sync.dma_start(out=h1b[:], in_=io["h1b"][:, :])
    h2kb = cpool.tile([MH, H], BF16, tag="h2kb")
    nc.sync.dma_start(out=h2kb[:], in_=io["h2kb"][:, :])
    vtile = cpool.tile([MH, NPR], BF16, tag="vtile")
    nc.sync.dma_start(out=vtile[:], in_=io["vtile"][:, :])
    labels = cpool.tile([128, NB], F32, tag="labels")
    nc.sync.dma_start(out=labels[:], in_=io["labels"][:, :])
    identb = cpool.tile([128, 128], BF16, tag="identb")
    nc.sync.dma_start(out=identb[:], in_=io["identb"][:, :])

    # ---- gathers: one per branch-piece, Q7 queue order = emission -----
    emb_t = cpool.tile([128, NB, EPAD], BF16, tag="emb_t")
    nc.gpsimd.dma_gather(
        emb_t[:], io["ttab"][:, :], idx_t[:], BC, BC, EPAD,
        queue_num=0, single_packet=False,
    )
    emb_c = cpool.tile([128, NB, EPAD], BF16, tag="emb_c")
    for s in range(4):
        nc.gpsimd.dma_gather(
            emb_c[:, 4 * s:4 * (s + 1), :], io[f"ctab{s}"][:, :],
            idx_c[:, (SEG // 16) * s:(SEG // 16) * (s + 1)], SEG, SEG, EPAD,
            queue_num=0, single_packet=False,
        )

    # ---- big weights on the Act queue (issue only; transfers overlap) --
    wrt = [cpool.tile([128, NPR], BF16, tag=f"wrt{j}", name=f"wrt{j}")
           for j in range(3)]
    wrg = [cpool.tile([128, NPR], BF16, tag=f"wrg{j}", name=f"wrg{j}")
           for j in range(3)]
    for j in range(3):
        nc.scalar.dma_start(out=wrt[j][:], in_=io["wrt"][128 * j:128 * (j + 1), :])
    for j in range(3):
        nc.scalar.dma_start(out=wrg[j][:], in_=io["wrg"][128 * j:128 * (j + 1), :])

    ones1 = cpool.tile([1, H], BF16, tag="ones1")
    nc.vector.memset(ones1[:], 1.0)
    ones128 = cpool.tile([128, 1], F32, tag="ones128")
    nc.vector.memset(ones128[:], 1.0)

    # ---- time MLP -> broadcast coefficient tiles ctile[c] --------------
    ctiles = []
    for c in range(NB):
        bcast = ptv.tile([H, 128], F32, tag="ptv", name=f"bcast{c}")
        nc.tensor.matmul(bcast[:], ones1[:], times[0:1, 128 * c:128 * (c + 1)],
                         start=True, stop=True)
        h1T = tvpool.tile([MH, 128], BF16, tag="h1T")
        nc.vector.memset(h1T[:], 1.0)
        nc.scalar.activation(h1T[0:H, :], bcast[:], AF.Tanh, bias=h1b[:],
                             scale=h1k[:])
        tvp = ptv.tile([H, 128], F32, tag="ptv", name=f"tvp{c}")
        nc.tensor.matmul(tvp[:], h2kb[:], h1T[:], start=True, stop=True)
        tvhT = tvpool.tile([MH, 128], BF16, tag="tvhT")
        nc.vector.memset(tvhT[:], 1.0)
        nc.scalar.activation(tvhT[0:H, :], tvp[:], AF.Tanh)
        cwp = pmm.tile([128, NPR], F32, tag="mp", name=f"cwp{c}")
        nc.tensor.matmul(cwp[:], tvhT[:], vtile[:], start=True, stop=True)
        ct = cpool.tile([128, NPR], BF16, tag=f"ct{c}", name=f"ct{c}")
        nc.vector.tensor_copy(ct[:], cwp[:])
        ctiles.append(ct)

    # ---- per-chunk branch contraction ---------------------------------
    def branch_mv(br, c, wr, emb, mv_out):
        if USE_DMA_TRANSPOSE:
            et3 = wpool.tile([128, 3, 128], BF16, tag=f"et3_{br}",
                             name=f"et3_{br}{c}")
            nc.sync.dma_start_transpose(et3[:], emb[:, c, :])
            lhs = [et3[:, j, :] for j in range(3)]
        else:
            lhs = []
            for j in range(3):
                tpp = ptr.tile([128, 128], BF16, tag="pt", name=f"pt{br}{c}{j}")
                nc.tensor.transpose(
                    tpp[:], emb[:, c, 128 * j:128 * (j + 1)], identb[:])
                et = wpool.tile([128, 128], BF16, tag=f"et{j}_{br}",
                                name=f"et{j}_{br}{c}")
                nc.vector.tensor_copy(et[:], tpp[:])
                lhs.append(et[:])
        mp = pmm.tile([128, NPR], F32, tag="mp", name=f"mp_{br}{c}")
        for j in range(3):
            nc.tensor.matmul(mp[:], lhs[j], wr[j][:], start=(j == 0),
                             stop=(j == 2))
        ms = wpool.tile([128, NPR], BF16, tag=f"ms_{br}", name=f"ms_{br}{c}")
        nc.scalar.copy(ms[:], mp[:])
        prod = wpool.tile([128, NPR], BF16, tag=f"prod_{br}",
                          name=f"prod_{br}{c}")
        nc.vector.tensor_mul(prod[:], ms[:], ctiles[c][:])
        nc.vector.reduce_sum(
            out=mv_out,
            in_=prod[:].rearrange("p (a k) -> p a k", k=R),
            axis=AX.X,
        )

    mvt = [cpool.tile([128, MH], F32, tag=f"mvt{c}", name=f"mvt{c}")
           for c in range(NB)]
    logits = cpool.tile([128, NB], F32, tag="logits")

    def do_c(c):
        mvc = wpool.tile([128, MH], F32, tag="mvc", name=f"mvc{c}")
        branch_mv("c", c, wrg, emb_c, mvc[:])
        junk = lpool.tile([128, MH], F32, tag="junk", name=f"junk{c}")
        nc.vector.tensor_mul(junk[:], mvt[c][:], mvc[:])
        nc.vector.reduce_sum(out=logits[:, c:c + 1], in_=junk[:], axis=AX.X)

    # interleave t/c chunks roughly in data-arrival order; a c chunk is
    # only emitted after its t chunk (the ttr reads mvt[c] on the same
    # DVE queue, so emission order must respect that dependency)
    ti = ci = 0
    for _ in range(6):
        branch_mv("t", ti, wrt, emb_t, mvt[ti][:])
        ti += 1
    while ti < NB or ci < NB:
        for _ in range(3):
            if ti < NB:
                branch_mv("t", ti, wrt, emb_t, mvt[ti][:])
                ti += 1
        for _ in range(4):
            if ci < NB and ci < ti:
                do_c(ci)
                ci += 1

    # ---- batched loss tail: softplus(l) - l*y over [128, NB] -----------
    ab = lpool.tile([128, NB], F32, tag="ab")
    nc.scalar.activation(ab[:], logits[:], AF.Abs)
    ex = lpool.tile([128, NB], F32, tag="ex")
    nc.scalar.activation(ex[:], ab[:], AF.Exp, scale=-1.0)
    l1p = lpool.tile([128, NB], F32, tag="l1p")
    nc.scalar.activation(l1p[:], ex[:], AF.Ln, bias=1.0)
    rl = lpool.tile([128, NB], F32, tag="rl")
    nc.scalar.activation(rl[:], logits[:], AF.Relu)
    sp = lpool.tile([128, NB], F32, tag="sp")
    nc.vector.tensor_add(sp[:], rl[:], l1p[:])
    ll = lpool.tile([128, NB], F32, tag="ll")
    nc.vector.tensor_mul(ll[:], logits[:], labels[:])
    dvec = lpool.tile([128, NB], F32, tag="dvec")
    nc.vector.tensor_sub(dvec[:], sp[:], ll[:])

    srow = cpool.tile([128, 1], F32, tag="srow")
    nc.vector.reduce_sum(out=srow[:], in_=dvec[:], axis=AX.X)
    fin = ptv.tile([1, 1], F32, tag="ptv", name="pfin")
    nc.tensor.matmul(fin[:], srow[:], ones128[:], start=True, stop=True)
    res = cpool.tile([1, 1], F32, tag="res")
    nc.scalar.copy(res[:], fin[:])
    nc.sync.dma_start(out=io["out"][:, :], in_=res[:])


_PROGRAM = None


def _get_program():
    global _PROGRAM
    if _PROGRAM is not None:
        return _PROGRAM
    nc = bacc.Bacc("TRN2", target_bir_lowering=False, debug=False,
                   num_devices=N_CORES)
    io = {
        "ttab": nc.dram_tensor("ttab", [W, EPAD], BF16, kind="ExternalInput").ap(),
        "wrt": nc.dram_tensor("wrt", [EPAD, NPR], BF16, kind="ExternalInput").ap(),
        "wrg": nc.dram_tensor("wrg", [EPAD, NPR], BF16, kind="ExternalInput").ap(),
        "vtile": nc.dram_tensor("vtile", [MH, NPR], BF16, kind="ExternalInput").ap(),
        "h2kb": nc.dram_tensor("h2kb", [MH, H], BF16, kind="ExternalInput").ap(),
        "h1k": nc.dram_tensor("h1k", [H, 1], F32, kind="ExternalInput").ap(),
        "h1b": nc.dram_tensor("h1b", [H, 1], F32, kind="ExternalInput").ap(),
        "identb": nc.dram_tensor("identb", [128, 128], BF16, kind="ExternalInput").ap(),
        "times": nc.dram_tensor("times", [1, BC], BF16, kind="ExternalInput").ap(),
        "labels": nc.dram_tensor("labels", [128, NB], F32, kind="ExternalInput").ap(),
        "idx_t": nc.dram_tensor("idx_t", [128, BC // 16], I16, kind="ExternalInput").ap(),
        "idx_c": nc.dram_tensor("idx_c", [128, BC // 16], I16, kind="ExternalInput").ap(),
        "out": nc.dram_tensor("out", [1, 1], F32, kind="ExternalOutput").ap(),
    }
    for s in range(4):
        io[f"ctab{s}"] = nc.dram_tensor(f"ctab{s}", [W, EPAD], BF16,
                                        kind="ExternalInput").ap()
    with tile.TileContext(nc) as tc:
        with ExitStack() as ctx:
            _build_kernel(ctx, tc, io)
    nc.compile()
    _PROGRAM = nc
    return nc


def _pad_table(tab):
    out = np.zeros((V, EPAD), dtype=ml_dtypes.bfloat16)
    out[:, :EMB] = np.asarray(tab).astype(ml_dtypes.bfloat16)
    out[:, EMB] = 1.0
    return out


def _precompute_weights(h1_k, h1_b, h2_k, h2_b, evoke_k, evoke_b,
                        last_k, last_b):
    """Affine rank-R tv basis + folded contraction weights (float64)."""
    h1_k = np.asarray(h1_k, np.float64)
    h1_b = np.asarray(h1_b, np.float64)
    h2_k = np.asarray(h2_k, np.float64)
    h2_b = np.asarray(h2_b, np.float64)
    g = np.linspace(0.0, 1.0, 8193, dtype=np.float64).reshape(-1, 1)
    h1g = np.tanh(g @ h1_k.reshape(1, H) + h1_b.reshape(H))
    tvg = np.tanh(h1g @ h2_k + h2_b.reshape(H))
    m = tvg.mean(0)
    _, _, vt = np.linalg.svd(tvg - m, full_matrices=False)
    v3 = vt[:R - 1].T                                   # [100, R-1]
    b_aff = np.concatenate([m.reshape(-1, 1), v3], 1)   # [100, R]
    vaff_h = np.zeros((MH, R))
    vaff_h[:H, 1:] = v3
    vaff_h[H, 0] = 1.0
    vaff_h[H, 1:] = -(m @ v3)

    evoke_pad = np.zeros((EPAD, H * H))
    evoke_pad[:EMB] = np.asarray(evoke_k, np.float64)
    evoke_pad[EMB] = np.asarray(evoke_b, np.float64)
    wr = (evoke_pad.reshape(EPAD * H, H) @ b_aff).reshape(EPAD, H, R)
    wr_full = np.zeros((EPAD, MH, R))
    wr_full[:, :H, :] = wr
    wr_full[EMB, H, 0] = 1.0
    lastkh = np.vstack([np.asarray(last_k, np.float64),
                        np.asarray(last_b, np.float64).reshape(1, EMB)])
    gh = lastkh @ lastkh.T
    wrgh = np.einsum('epk,pq->eqk', wr_full, gh)

    wrt = wr_full.reshape(EPAD, NPR).astype(ml_dtypes.bfloat16)
    wrg = wrgh.reshape(EPAD, NPR).astype(ml_dtypes.bfloat16)
    vtile = np.tile(vaff_h, (1, MH)).astype(ml_dtypes.bfloat16)
    h2kb = np.vstack([h2_k, h2_b.reshape(1, H)]).astype(ml_dtypes.bfloat16)
    h1kc = h1_k.reshape(1, H).T.astype(np.float32).copy()
    h1bc = h1_b.reshape(H, 1).astype(np.float32).copy()
    return wrt, wrg, vtile, h2kb, h1kc, h1bc


def build_in_maps(targets, contexts, times, labels, targetemb, contextemb,
                  h1_k, h1_b, h2_k, h2_b, evoke_k, evoke_b, last_k, last_b):
    global LAST_PERMS
    ttab = _pad_table(targetemb)
    ctab = _pad_table(contextemb)
    wrt, wrg, vtile, h2kb, h1kc, h1bc = _precompute_weights(
        h1_k, h1_b, h2_k, h2_b, evoke_k, evoke_b, last_k, last_b)
    identb = np.eye(128, dtype=ml_dtypes.bfloat16)
    targets = np.asarray(targets).astype(np.int64)
    contexts = np.asarray(contexts).astype(np.int64)
    times = np.asarray(times).astype(np.float32)
    labels = np.asarray(labels).astype(np.float32)

    order_t = np.argsort(targets, kind="stable")
    in_maps = []
    perms = []
    for k in range(N_CORES):
        i_k = order_t[k * BC:(k + 1) * BC]
        j_k = i_k[np.argsort(contexts[i_k], kind="stable")]
        perms.append(j_k)
        tv = targets[j_k]
        cv = contexts[j_k]
        off_t = min(int(tv.min()), V - W)
        t_loc = tv - off_t
        assert t_loc.min() >= 0 and t_loc.max() < W, "t window overflow"
        m = {
            "ttab": ttab[off_t:off_t + W],
            "wrt": wrt, "wrg": wrg, "vtile": vtile, "h2kb": h2kb,
            "h1k": h1kc, "h1b": h1bc, "identb": identb,
            "times": times[j_k].astype(ml_dtypes.bfloat16).reshape(1, BC),
            "labels": labels[j_k].reshape(NB, 128).T.copy(),
            "idx_t": _wrap16(t_loc),
        }
        c_loc = np.empty(BC, dtype=np.int64)
        for s in range(4):
            seg = cv[SEG * s:SEG * (s + 1)]
            base = min(int(seg[0]), V - W)
            loc = seg - base
            assert loc.min() >= 0 and loc.max() < W, "c window overflow"
            c_loc[SEG * s:SEG * (s + 1)] = loc
            m[f"ctab{s}"] = ctab[base:base + W]
        m["idx_c"] = _wrap16(c_loc)
        in_maps.append(m)
    LAST_PERMS = perms
    return in_maps


def kernel(**inputs) -> np.ndarray:
    nc = _get_program()
    in_maps = build_in_maps(**inputs)
    r = run_bass_kernel_spmd(nc, in_maps, list(range(N_CORES)))
    total = np.float64(0.0)
    for m in r.results:
        total += np.float64(m["out"][0, 0])
    return np.float32(total / B)


# revision 9
# speedup vs baseline: 3.0801x; 1.1093x over previous
"""Trainium2 Bass kernel for nn_DiffTime (embedding_lookup, 8 NeuronCores).

Reference computation:
    h1 = tanh(times * h1_k + h1_b)            [B, 100]
    tv = tanh(h1 @ h2_k + h2_b)               [B, 100]
    mat_x = (emb_x @ evoke_k + evoke_b)       [B, 100p, 100h]   (x in {target, context})
    mv_x = einsum('bph,bh->bp', mat_x, tv)    [B, 100]
    vect_x = mv_x @ last_k + last_b           [B, 300]
    logits = sum(vect_t * vect_c, -1)         [B]
    out = mean(softplus(logits) - logits * labels)

Kernel strategy (data-parallel, 2048 items/core, no collectives):

* tv rows lie on a smooth 1-D curve in R^100; an affine rank-4 basis
  (mean + 3 SVD directions of the centered curve, c0 == 1 by a
  homogeneous-coordinate trick) reproduces the final loss to ~4e-6.
  The h-contraction is folded into the weights on the host:
  Wr[e,(p,k)] = sum_h evoke_pad[e,p*100+h]*B_aff[h,k], so the kernel
  contracts emb (384-padded, homogeneous col 300 == 1) against a
  [384, 404] matrix and reduces over k=4 with a broadcast coefficient
  tile.  The Gram matrix Gh = lastkh @ lastkh.T (which turns the two
  [B,300] branch vectors into a [101]x[101] bilinear form) is folded
  into the context branch weights as well, so logits are a single
  fused multiply-reduce of the two [128,101] mv tiles.

* Gathers are single-stage on both branches (no scratch / realign):
  - batch items are assigned to cores by a global argsort of targets,
    so each core's target rows fall inside one 32768-row table window
    (span ~12.5k) => one 2048-row int16 dma_gather from a per-core
    window slice fed as input;
  - within each core, items are processed in context-sorted order
    (the loss is an order-invariant mean, so any processing order
    works as long as times/labels/indices are permuted consistently);
    the sorted contexts are cut at ranks 512/1024/1536 and gathered
    with four 512-row dma_gathers from per-core percentile windows
    (span of 512 sorted uniform draws ~26k < 32768).

* emb transposes ([b,e] -> [e,b] for the PE contraction) use the XBAR
  dma_start_transpose (SBUF->SBUF, [128,384] -> [128,3,128]) on the
  otherwise-idle SP queue instead of PE transposes + PSUM evictions.
"""

import sys

for _p in ("/opt/trn_rl_repo", "/opt/trn_rl_repo/concourse"):
    if _p not in sys.path:
        sys.path.insert(0, _p)

from contextlib import ExitStack

import ml_dtypes
import numpy as np

import concourse.bacc as bacc
import concourse.bass as bass
import concourse.tile as tile
from concourse import mybir
from concourse.bass_utils import run_bass_kernel_spmd

F32 = mybir.dt.float32
BF16 = mybir.dt.bfloat16
I16 = mybir.dt.int16
AF = mybir.ActivationFunctionType
AX = mybir.AxisListType
OP = mybir.AluOpType

N_CORES = 8
B = 16384
BC = B // N_CORES          # 2048 batch items per core
NB = BC // 128             # 16 chunks of 128 batch rows
V = 100000
EMB = 300
EPAD = 384                 # padded embedding row (col 300 = 1.0, rest 0)
H = 100
MH = H + 1                 # homogeneous mv size
R = 4                      # affine tv-basis rank (c0 == 1)
NPR = MH * R               # 404 contracted columns
W = 32768                  # per-core table window (int16-addressable)
SEG = 512                  # context gather piece (4 x 512 = 2048)

USE_DMA_TRANSPOSE = True

LAST_PERMS = None          # debug: per-core batch permutation of last build


def _wrap16(v):
    """int16 index array -> dma_gather SBUF layout [128, len//16]."""
    v = np.asarray(v, dtype=np.int16)
    a = v.reshape(-1, 16).T
    return np.tile(a, (8, 1))


def _build_kernel(ctx: ExitStack, tc: "tile.TileContext", io: dict):
    nc = tc.nc

    cpool = ctx.enter_context(tc.tile_pool(name="const", bufs=1))
    wpool = ctx.enter_context(tc.tile_pool(name="work", bufs=4))
    tvpool = ctx.enter_context(tc.tile_pool(name="tvp", bufs=3))
    lpool = ctx.enter_context(tc.tile_pool(name="loss", bufs=2))
    pmm = ctx.enter_context(tc.tile_pool(name="pmm", bufs=3, space="PSUM"))
    ptv = ctx.enter_context(tc.tile_pool(name="ptv", bufs=2, space="PSUM"))
    if not USE_DMA_TRANSPOSE:
        ptr = ctx.enter_context(tc.tile_pool(name="ptr", bufs=3, space="PSUM"))

    # ---- small resident constants (SP queue: these come first so the
    # gathers, which only need the index tiles, start immediately) ------
    idx_t = cpool.tile([128, BC // 16], I16, tag="idx_t")
    nc.sync.dma_start(out=idx_t[:], in_=io["idx_t"][:, :])
    idx_c = cpool.tile([128, BC // 16], I16, tag="idx_c")
    nc.sync.dma_start(out=idx_c[:], in_=io["idx_c"][:, :])

    # ---- gathers: one per branch-piece, Q7 queue order = emission -----
    emb_t = cpool.tile([128, NB, EPAD], BF16, tag="emb_t")
    nc.gpsimd.dma_gather(
        emb_t[:], io["ttab"][:, :], idx_t[:], BC, BC, EPAD,
        queue_num=0, single_packet=False,
    )
    emb_c = cpool.tile([128, NB, EPAD], BF16, tag="emb_c")
    for s in range(4):
        nc.gpsimd.dma_gather(
            emb_c[:, 4 * s:4 * (s + 1), :], io[f"ctab{s}"][:, :],
            idx_c[:, (SEG // 16) * s:(SEG // 16) * (s + 1)], SEG, SEG, EPAD,
            queue_num=(s + 1) % 4, single_packet=False,
        )

    # ---- remaining constants: small ones + big weights, split SP/Act ----
    times = cpool.tile([1, BC], BF16, tag="times")
    nc.sync.dma_start(out=times[:], in_=io["times"][:, :])
    h1k = cpool.tile([H, 1], F32, tag="h1k")
    nc.scalar.dma_start(out=h1k[:], in_=io["h1k"][:, :])
    h1b = cpool.tile([H, 1], F32, tag="h1b")
    nc.scalar.dma_start(out=h1b[:], in_=io["h1b"][:, :])
    h2k = cpool.tile([H, H], BF16, tag="h2k")
    nc.sync.dma_start(out=h2k[:], in_=io["h2k"][:, :])
    h2b = cpool.tile([H, 1], F32, tag="h2b")
    nc.scalar.dma_start(out=h2b[:], in_=io["h2b"][:, :])
    vtile = cpool.tile([MH, NPR], BF16, tag="vtile")
    nc.sync.dma_start(out=vtile[:], in_=io["vtile"][:, :])
    labels = cpool.tile([128, NB], F32, tag="labels")
    nc.scalar.dma_start(out=labels[:], in_=io["labels"][:, :])
    identb = cpool.tile([128, 128], BF16, tag="identb")
    nc.scalar.dma_start(out=identb[:], in_=io["identb"][:, :])
    wrt = [cpool.tile([128, NPR], BF16, tag=f"wrt{j}", name=f"wrt{j}")
           for j in range(3)]
    wrg = [cpool.tile([128, NPR], BF16, tag=f"wrg{j}", name=f"wrg{j}")
           for j in range(3)]
    for j in range(3):
        nc.sync.dma_start(out=wrt[j][:], in_=io["wrt"][128 * j:128 * (j + 1), :])
    for j in range(3):
        nc.scalar.dma_start(out=wrg[j][:], in_=io["wrg"][128 * j:128 * (j + 1), :])

    ones1 = cpool.tile([1, H], BF16, tag="ones1")
    nc.vector.memset(ones1[:], 1.0)
    ones128 = cpool.tile([128, 1], F32, tag="ones128")
    nc.vector.memset(ones128[:], 1.0)

    # ---- time MLP -> broadcast coefficient tiles ctile[c] --------------
    tvh_bufs = [cpool.tile([MH, 128], BF16, tag=f"tvhb{i}", name=f"tvhb{i}")
                for i in range(3)]
    for i in range(3):
        nc.vector.memset(tvh_bufs[i][:], 1.0)
    ctiles = []
    for c in range(NB):
        bcast = ptv.tile([H, 128], F32, tag="ptv", name=f"bcast{c}")
        nc.tensor.matmul(bcast[:], ones1[:], times[0:1, 128 * c:128 * (c + 1)],
                         start=True, stop=True)
        h1T = tvpool.tile([H, 128], BF16, tag="h1T")
        nc.scalar.activation(h1T[:], bcast[:], AF.Tanh, bias=h1b[:],
                             scale=h1k[:])
        tvp = ptv.tile([H, 128], F32, tag="ptv", name=f"tvp{c}")
        nc.tensor.matmul(tvp[:], h2k[:], h1T[:], start=True, stop=True)
        tvhT = tvh_bufs[c % 3]
        nc.scalar.activation(tvhT[0:H, :], tvp[:], AF.Tanh, bias=h2b[:])
        cwp = pmm.tile([128, NPR], F32, tag="mp", name=f"cwp{c}")
        nc.tensor.matmul(cwp[:], tvhT[:], vtile[:], start=True, stop=True)
        ct = cpool.tile([128, NPR], BF16, tag=f"ct{c}", name=f"ct{c}")
        nc.vector.tensor_copy(ct[:], cwp[:])
        ctiles.append(ct)

    # ---- per-chunk branch contraction ---------------------------------
    et12 = {}

    def group_transpose(br, g, emb):
        t = wpool.tile([128, 12, 128], BF16, tag=f"et12_{br}",
                       name=f"et12_{br}{g}")
        nc.sync.dma_start_transpose(
            t[:], emb[:, 4 * g:4 * (g + 1), :].rearrange("p c e -> p (c e)"))
        et12[br, g] = t

    def branch_mv(br, c, wr, emb, mv_out):
        if USE_DMA_TRANSPOSE:
            if (br, c // 4) not in et12:
                group_transpose(br, c // 4, emb)
            t = et12[br, c // 4]
            lhs = [t[:, (c % 4) * 3 + j, :] for j in range(3)]
        else:
            lhs = []
            for j in range(3):
                tpp = ptr.tile([128, 128], BF16, tag="pt", name=f"pt{br}{c}{j}")
                nc.tensor.transpose(
                    tpp[:], emb[:, c, 128 * j:128 * (j + 1)], identb[:])
                et = wpool.tile([128, 128], BF16, tag=f"et{j}_{br}",
                                name=f"et{j}_{br}{c}")
                nc.vector.tensor_copy(et[:], tpp[:])
                lhs.append(et[:])
        mp = pmm.tile([128, NPR], F32, tag="mp", name=f"mp_{br}{c}")
        for j in range(3):
            nc.tensor.matmul(mp[:], lhs[j], wr[j][:], start=(j == 0),
                             stop=(j == 2))
        ms = wpool.tile([128, NPR], BF16, tag=f"ms_{br}", name=f"ms_{br}{c}")
        nc.scalar.copy(ms[:], mp[:])
        prod = wpool.tile([128, NPR], BF16, tag=f"prod_{br}",
                          name=f"prod_{br}{c}")
        nc.vector.tensor_mul(prod[:], ms[:], ctiles[c][:])
        nc.vector.reduce_sum(
            out=mv_out,
            in_=prod[:].rearrange("p (a k) -> p a k", k=R),
            axis=AX.X,
        )

    mvt = [cpool.tile([128, MH], F32, tag=f"mvt{c}", name=f"mvt{c}")
           for c in range(NB)]
    logits = cpool.tile([128, NB], F32, tag="logits")

    def do_c(c):
        mvc = wpool.tile([128, MH], F32, tag="mvc", name=f"mvc{c}")
        branch_mv("c", c, wrg, emb_c, mvc[:])
        junk = lpool.tile([128, MH], F32, tag="junk", name=f"junk{c}")
        nc.vector.tensor_mul(junk[:], mvt[c][:], mvc[:])
        nc.vector.reduce_sum(out=logits[:, c:c + 1], in_=junk[:], axis=AX.X)

    # interleave t/c chunks roughly in data-arrival order; a c chunk is
    # only emitted after its t chunk (the ttr reads mvt[c] on the same
    # DVE queue, so emission order must respect that dependency)
    ti = ci = 0
    for _ in range(6):
        branch_mv("t", ti, wrt, emb_t, mvt[ti][:])
        ti += 1
    while ti < NB or ci < NB:
        for _ in range(3):
            if ti < NB:
                branch_mv("t", ti, wrt, emb_t, mvt[ti][:])
                ti += 1
        for _ in range(4):
            if ci < NB and ci < ti:
                do_c(ci)
                ci += 1

    # ---- batched loss tail: softplus(l) - l*y over [128, NB] -----------
    ab = lpool.tile([128, NB], F32, tag="ab")
    nc.scalar.activation(ab[:], logits[:], AF.Abs)
    ex = lpool.tile([128, NB], F32, tag="ex")
    nc.scalar.activation(ex[:], ab[:], AF.Exp, scale=-1.0)
    l1p = lpool.tile([128, NB], F32, tag="l1p")
    nc.scalar.activation(l1p[:], ex[:], AF.Ln, bias=1.0)
    rl = lpool.tile([128, NB], F32, tag="rl")
    nc.scalar.activation(rl[:], logits[:], AF.Relu)
    sp = lpool.tile([128, NB], F32, tag="sp")
    nc.vector.tensor_add(sp[:], rl[:], l1p[:])
    ll = lpool.tile([128, NB], F32, tag="ll")
    nc.vector.tensor_mul(ll[:], logits[:], labels[:])
    dvec = lpool.tile([128, NB], F32, tag="dvec")
    nc.vector.tensor_sub(dvec[:], sp[:], ll[:])

    srow = cpool.tile([128, 1], F32, tag="srow")
    nc.vector.reduce_sum(out=srow[:], in_=dvec[:], axis=AX.X)
    fin = ptv.tile([1, 1], F32, tag="ptv", name="pfin")
    nc.tensor.matmul(fin[:], srow[:], ones128[:], start=True, stop=True)
    res = cpool.tile([1, 1], F32, tag="res")
    nc.scalar.copy(res[:], fin[:])
    nc.sync.dma_start(out=io["out"][:, :], in_=res[:])


_PROGRAM = None


def _get_program():
    global _PROGRAM
    if _PROGRAM is not None:
        return _PROGRAM
    nc = bacc.Bacc("TRN2", target_bir_lowering=False, debug=False,
                   num_devices=N_CORES, num_swdge_queues=4)
    io = {
        "ttab": nc.dram_tensor("ttab", [W, EPAD], BF16, kind="ExternalInput").ap(),
        "wrt": nc.dram_tensor("wrt", [EPAD, NPR], BF16, kind="ExternalInput").ap(),
        "wrg": nc.dram_tensor("wrg", [EPAD, NPR], BF16, kind="ExternalInput").ap(),
        "vtile": nc.dram_tensor("vtile", [MH, NPR], BF16, kind="ExternalInput").ap(),
        "h2k": nc.dram_tensor("h2k", [H, H], BF16, kind="ExternalInput").ap(),
        "h2b": nc.dram_tensor("h2b", [H, 1], F32, kind="ExternalInput").ap(),
        "h1k": nc.dram_tensor("h1k", [H, 1], F32, kind="ExternalInput").ap(),
        "h1b": nc.dram_tensor("h1b", [H, 1], F32, kind="ExternalInput").ap(),
        "identb": nc.dram_tensor("identb", [128, 128], BF16, kind="ExternalInput").ap(),
        "times": nc.dram_tensor("times", [1, BC], BF16, kind="ExternalInput").ap(),
        "labels": nc.dram_tensor("labels", [128, NB], F32, kind="ExternalInput").ap(),
        "idx_t": nc.dram_tensor("idx_t", [128, BC // 16], I16, kind="ExternalInput").ap(),
        "idx_c": nc.dram_tensor("idx_c", [128, BC // 16], I16, kind="ExternalInput").ap(),
        "out": nc.dram_tensor("out", [1, 1], F32, kind="ExternalOutput").ap(),
    }
    for s in range(4):
        io[f"ctab{s}"] = nc.dram_tensor(f"ctab{s}", [W, EPAD], BF16,
                                        kind="ExternalInput").ap()
    with tile.TileContext(nc) as tc:
        with ExitStack() as ctx:
            _build_kernel(ctx, tc, io)
    nc.compile()
    _PROGRAM = nc
    return nc


def _pad_table(tab):
    out = np.zeros((V, EPAD), dtype=ml_dtypes.bfloat16)
    out[:, :EMB] = np.asarray(tab).astype(ml_dtypes.bfloat16)
    out[:, EMB] = 1.0
    return out


def _precompute_weights(h1_k, h1_b, h2_k, h2_b, evoke_k, evoke_b,
                        last_k, last_b):
    """Affine rank-R tv basis + folded contraction weights (float64)."""
    h1_k = np.asarray(h1_k, np.float64)
    h1_b = np.asarray(h1_b, np.float64)
    h2_k = np.asarray(h2_k, np.float64)
    h2_b = np.asarray(h2_b, np.float64)
    g = np.linspace(0.0, 1.0, 8193, dtype=np.float64).reshape(-1, 1)
    h1g = np.tanh(g @ h1_k.reshape(1, H) + h1_b.reshape(H))
    tvg = np.tanh(h1g @ h2_k + h2_b.reshape(H))
    m = tvg.mean(0)
    _, _, vt = np.linalg.svd(tvg - m, full_matrices=False)
    v3 = vt[:R - 1].T                                   # [100, R-1]
    b_aff = np.concatenate([m.reshape(-1, 1), v3], 1)   # [100, R]
    vaff_h = np.zeros((MH, R))
    vaff_h[:H, 1:] = v3
    vaff_h[H, 0] = 1.0
    vaff_h[H, 1:] = -(m @ v3)

    evoke_pad = np.zeros((EPAD, H * H))
    evoke_pad[:EMB] = np.asarray(evoke_k, np.float64)
    evoke_pad[EMB] = np.asarray(evoke_b, np.float64)
    wr = (evoke_pad.reshape(EPAD * H, H) @ b_aff).reshape(EPAD, H, R)
    wr_full = np.zeros((EPAD, MH, R))
    wr_full[:, :H, :] = wr
    wr_full[EMB, H, 0] = 1.0
    lastkh = np.vstack([np.asarray(last_k, np.float64),
                        np.asarray(last_b, np.float64).reshape(1, EMB)])
    gh = lastkh @ lastkh.T
    wrgh = np.einsum('epk,pq->eqk', wr_full, gh)

    wrt = wr_full.reshape(EPAD, NPR).astype(ml_dtypes.bfloat16)
    wrg = wrgh.reshape(EPAD, NPR).astype(ml_dtypes.bfloat16)
    vtile = np.tile(vaff_h, (1, MH)).astype(ml_dtypes.bfloat16)
    h2kc = h2_k.astype(ml_dtypes.bfloat16)
    h2bc = h2_b.reshape(H, 1).astype(np.float32).copy()
    h1kc = h1_k.reshape(1, H).T.astype(np.float32).copy()
    h1bc = h1_b.reshape(H, 1).astype(np.float32).copy()
    return wrt, wrg, vtile, h2kc, h2bc, h1kc, h1bc


def build_in_maps(targets, contexts, times, labels, targetemb, contextemb,
                  h1_k, h1_b, h2_k, h2_b, evoke_k, evoke_b, last_k, last_b):
    global LAST_PERMS
    ttab = _pad_table(targetemb)
    ctab = _pad_table(contextemb)
    wrt, wrg, vtile, h2kc, h2bc, h1kc, h1bc = _precompute_weights(
        h1_k, h1_b, h2_k, h2_b, evoke_k, evoke_b, last_k, last_b)
    identb = np.eye(128, dtype=ml_dtypes.bfloat16)
    targets = np.asarray(targets).astype(np.int64)
    contexts = np.asarray(contexts).astype(np.int64)
    times = np.asarray(times).astype(np.float32)
    labels = np.asarray(labels).astype(np.float32)

    order_t = np.argsort(targets, kind="stable")
    in_maps = []
    perms = []
    for k in range(N_CORES):
        i_k = order_t[k * BC:(k + 1) * BC]
        j_k = i_k[np.argsort(contexts[i_k], kind="stable")]
        perms.append(j_k)
        tv = targets[j_k]
        cv = contexts[j_k]
        off_t = min(int(tv.min()), V - W)
        t_loc = tv - off_t
        assert t_loc.min() >= 0 and t_loc.max() < W, "t window overflow"
        m = {
            "ttab": ttab[off_t:off_t + W],
            "wrt": wrt, "wrg": wrg, "vtile": vtile,
            "h2k": h2kc, "h2b": h2bc,
            "h1k": h1kc, "h1b": h1bc, "identb": identb,
            "times": times[j_k].astype(ml_dtypes.bfloat16).reshape(1, BC),
            "labels": labels[j_k].reshape(NB, 128).T.copy(),
            "idx_t": _wrap16(t_loc),
        }
        c_loc = np.empty(BC, dtype=np.int64)
        for s in range(4):
            seg = cv[SEG * s:SEG * (s + 1)]
            base = min(int(seg[0]), V - W)
            loc = seg - base
            assert loc.min() >= 0 and loc.max() < W, "c window overflow"
            c_loc[SEG * s:SEG * (s + 1)] = loc
            m[f"ctab{s}"] = ctab[base:base + W]
        m["idx_c"] = _wrap16(c_loc)
        in_maps.append(m)
    LAST_PERMS = perms
    return in_maps


def kernel(**inputs) -> np.ndarray:
    nc = _get_program()
    in_maps = build_in_maps(**inputs)
    r = run_bass_kernel_spmd(nc, in_maps, list(range(N_CORES)))
    total = np.float64(0.0)
    for m in r.results:
        total += np.float64(m["out"][0, 0])
    return np.float32(total / B)


# revision 11
# speedup vs baseline: 3.0920x; 1.0039x over previous
"""Trainium2 Bass kernel for nn_DiffTime (embedding_lookup, 8 NeuronCores).

Reference computation:
    h1 = tanh(times * h1_k + h1_b)            [B, 100]
    tv = tanh(h1 @ h2_k + h2_b)               [B, 100]
    mat_x = (emb_x @ evoke_k + evoke_b)       [B, 100p, 100h]   (x in {target, context})
    mv_x = einsum('bph,bh->bp', mat_x, tv)    [B, 100]
    vect_x = mv_x @ last_k + last_b           [B, 300]
    logits = sum(vect_t * vect_c, -1)         [B]
    out = mean(softplus(logits) - logits * labels)

Kernel strategy (data-parallel, 2048 items/core, no collectives):

* tv rows lie on a smooth 1-D curve in R^100; an affine rank-4 basis
  (mean + 3 SVD directions of the centered curve, c0 == 1 by a
  homogeneous-coordinate trick) reproduces the final loss to ~4e-6.
  The h-contraction is folded into the weights on the host:
  Wr[e,(p,k)] = sum_h evoke_pad[e,p*100+h]*B_aff[h,k], so the kernel
  contracts emb (384-padded, homogeneous col 300 == 1) against a
  [384, 404] matrix and reduces over k=4 with a broadcast coefficient
  tile.  The Gram matrix Gh = lastkh @ lastkh.T (which turns the two
  [B,300] branch vectors into a [101]x[101] bilinear form) is folded
  into the context branch weights as well, so logits are a single
  fused multiply-reduce of the two [128,101] mv tiles.

* Gathers are single-stage on both branches (no scratch / realign):
  - batch items are assigned to cores by a global argsort of targets,
    so each core's target rows fall inside one 32768-row table window
    (span ~12.5k) => one 2048-row int16 dma_gather from a per-core
    window slice fed as input;
  - within each core, items are processed in context-sorted order
    (the loss is an order-invariant mean, so any processing order
    works as long as times/labels/indices are permuted consistently);
    the sorted contexts are cut at ranks 512/1024/1536 and gathered
    with four 512-row dma_gathers from per-core percentile windows
    (span of 512 sorted uniform draws ~26k < 32768).

* emb transposes ([b,e] -> [e,b] for the PE contraction) use the XBAR
  dma_start_transpose (SBUF->SBUF, [128,384] -> [128,3,128]) on the
  otherwise-idle SP queue instead of PE transposes + PSUM evictions.
"""

import sys

for _p in ("/opt/trn_rl_repo", "/opt/trn_rl_repo/concourse"):
    if _p not in sys.path:
        sys.path.insert(0, _p)

from contextlib import ExitStack

import ml_dtypes
import numpy as np

import concourse.bacc as bacc
import concourse.bass as bass
import concourse.tile as tile
from concourse import mybir
from concourse.bass_utils import run_bass_kernel_spmd

F32 = mybir.dt.float32
BF16 = mybir.dt.bfloat16
I16 = mybir.dt.int16
AF = mybir.ActivationFunctionType
AX = mybir.AxisListType
OP = mybir.AluOpType

N_CORES = 8
B = 16384
BC = B // N_CORES          # 2048 batch items per core
NB = BC // 128             # 16 chunks of 128 batch rows
V = 100000
EMB = 300
EPAD = 384                 # padded embedding row (col 300 = 1.0, rest 0)
H = 100
MH = H + 1                 # homogeneous mv size
R = 4                      # affine tv-basis rank (c0 == 1)
NPR = MH * R               # 404 contracted columns
W = 32768                  # per-core table window (int16-addressable)
SEG = 512                  # context gather piece (4 x 512 = 2048)

USE_DMA_TRANSPOSE = True

LAST_PERMS = None          # debug: per-core batch permutation of last build


def _wrap16(v):
    """int16 index array -> dma_gather SBUF layout [128, len//16]."""
    v = np.asarray(v, dtype=np.int16)
    a = v.reshape(-1, 16).T
    return np.tile(a, (8, 1))


def _build_kernel(ctx: ExitStack, tc: "tile.TileContext", io: dict):
    nc = tc.nc

    cpool = ctx.enter_context(tc.tile_pool(name="const", bufs=1))
    wpool = ctx.enter_context(tc.tile_pool(name="work", bufs=4))
    tvpool = ctx.enter_context(tc.tile_pool(name="tvp", bufs=3))
    lpool = ctx.enter_context(tc.tile_pool(name="loss", bufs=2))
    pmm = ctx.enter_context(tc.tile_pool(name="pmm", bufs=3, space="PSUM"))
    ptv = ctx.enter_context(tc.tile_pool(name="ptv", bufs=2, space="PSUM"))
    if not USE_DMA_TRANSPOSE:
        ptr = ctx.enter_context(tc.tile_pool(name="ptr", bufs=3, space="PSUM"))

    # ---- small resident constants (SP queue: these come first so the
    # gathers, which only need the index tiles, start immediately) ------
    idx_t = cpool.tile([128, BC // 16], I16, tag="idx_t")
    nc.sync.dma_start(out=idx_t[:], in_=io["idx_t"][:, :])
    idx_c = cpool.tile([128, BC // 16], I16, tag="idx_c")
    nc.sync.dma_start(out=idx_c[:], in_=io["idx_c"][:, :])

    # ---- gathers: one per branch-piece, Q7 queue order = emission -----
    emb_t = cpool.tile([128, NB, EPAD], BF16, tag="emb_t")
    emb_c = cpool.tile([128, NB, EPAD], BF16, tag="emb_c")
    qn = 0
    for s in range(4):
        nc.gpsimd.dma_gather(
            emb_t[:, 4 * s:4 * (s + 1), :], io["ttab"][:, :],
            idx_t[:, (SEG // 16) * s:(SEG // 16) * (s + 1)], SEG, SEG, EPAD,
            queue_num=qn % 4, single_packet=False,
        )
        qn += 1
        nc.gpsimd.dma_gather(
            emb_c[:, 4 * s:4 * (s + 1), :], io[f"ctab{s}"][:, :],
            idx_c[:, (SEG // 16) * s:(SEG // 16) * (s + 1)], SEG, SEG, EPAD,
            queue_num=qn % 4, single_packet=False,
        )
        qn += 1

    # ---- remaining constants: small ones + big weights, split SP/Act ----
    times = cpool.tile([1, BC], BF16, tag="times")
    nc.sync.dma_start(out=times[:], in_=io["times"][:, :])
    h1k = cpool.tile([H, 1], F32, tag="h1k")
    nc.scalar.dma_start(out=h1k[:], in_=io["h1k"][:, :])
    h1b = cpool.tile([H, 1], F32, tag="h1b")
    nc.scalar.dma_start(out=h1b[:], in_=io["h1b"][:, :])
    h2k = cpool.tile([H, H], BF16, tag="h2k")
    nc.sync.dma_start(out=h2k[:], in_=io["h2k"][:, :])
    h2b = cpool.tile([H, 1], F32, tag="h2b")
    nc.scalar.dma_start(out=h2b[:], in_=io["h2b"][:, :])
    vtile = cpool.tile([MH, NPR], BF16, tag="vtile")
    nc.sync.dma_start(out=vtile[:], in_=io["vtile"][:, :])
    labels = cpool.tile([128, NB], F32, tag="labels")
    nc.scalar.dma_start(out=labels[:], in_=io["labels"][:, :])
    identb = cpool.tile([128, 128], BF16, tag="identb")
    nc.scalar.dma_start(out=identb[:], in_=io["identb"][:, :])
    wrt = [cpool.tile([128, NPR], BF16, tag=f"wrt{j}", name=f"wrt{j}")
           for j in range(3)]
    wrg = [cpool.tile([128, NPR], BF16, tag=f"wrg{j}", name=f"wrg{j}")
           for j in range(3)]
    for j in range(3):
        nc.sync.dma_start(out=wrt[j][:], in_=io["wrt"][128 * j:128 * (j + 1), :])
    for j in range(3):
        nc.scalar.dma_start(out=wrg[j][:], in_=io["wrg"][128 * j:128 * (j + 1), :])


    ones1 = cpool.tile([1, H], BF16, tag="ones1")
    nc.vector.memset(ones1[:], 1.0)
    ones128 = cpool.tile([128, 1], F32, tag="ones128")
    nc.vector.memset(ones128[:], 1.0)

    # ---- time MLP -> broadcast coefficient tiles ctile[c] --------------
    tvh_bufs = [cpool.tile([MH, 128], BF16, tag=f"tvhb{i}", name=f"tvhb{i}")
                for i in range(3)]
    for i in range(3):
        nc.vector.memset(tvh_bufs[i][:], 1.0)
    ctiles = []
    for c in range(NB):
        bcast = ptv.tile([H, 128], F32, tag="ptv", name=f"bcast{c}")
        nc.tensor.matmul(bcast[:], ones1[:], times[0:1, 128 * c:128 * (c + 1)],
                         start=True, stop=True)
        h1T = tvpool.tile([H, 128], BF16, tag="h1T")
        nc.scalar.activation(h1T[:], bcast[:], AF.Tanh, bias=h1b[:],
                             scale=h1k[:])
        tvp = ptv.tile([H, 128], F32, tag="ptv", name=f"tvp{c}")
        nc.tensor.matmul(tvp[:], h2k[:], h1T[:], start=True, stop=True)
        tvhT = tvh_bufs[c % 3]
        nc.scalar.activation(tvhT[0:H, :], tvp[:], AF.Tanh, bias=h2b[:])
        cwp = pmm.tile([128, NPR], F32, tag="mp", name=f"cwp{c}")
        nc.tensor.matmul(cwp[:], tvhT[:], vtile[:], start=True, stop=True)
        ct = cpool.tile([128, NPR], BF16, tag=f"ct{c}", name=f"ct{c}")
        nc.vector.tensor_copy(ct[:], cwp[:])
        ctiles.append(ct)

    # ---- per-chunk branch contraction ---------------------------------
    et12 = {}

    def group_transpose(br, g, emb):
        t = wpool.tile([128, 12, 128], BF16, tag=f"et12_{br}",
                       name=f"et12_{br}{g}")
        nc.sync.dma_start_transpose(
            t[:], emb[:, 4 * g:4 * (g + 1), :].rearrange("p c e -> p (c e)"))
        et12[br, g] = t

    def branch_mv(br, c, wr, emb, mv_out):
        if USE_DMA_TRANSPOSE:
            if (br, c // 4) not in et12:
                group_transpose(br, c // 4, emb)
            t = et12[br, c // 4]
            lhs = [t[:, (c % 4) * 3 + j, :] for j in range(3)]
        else:
            lhs = []
            for j in range(3):
                tpp = ptr.tile([128, 128], BF16, tag="pt", name=f"pt{br}{c}{j}")
                nc.tensor.transpose(
                    tpp[:], emb[:, c, 128 * j:128 * (j + 1)], identb[:])
                et = wpool.tile([128, 128], BF16, tag=f"et{j}_{br}",
                                name=f"et{j}_{br}{c}")
                nc.vector.tensor_copy(et[:], tpp[:])
                lhs.append(et[:])
        mp = pmm.tile([128, NPR], F32, tag="mp", name=f"mp_{br}{c}")
        for j in range(3):
            nc.tensor.matmul(mp[:], lhs[j], wr[j][:], start=(j == 0),
                             stop=(j == 2))
        ms = wpool.tile([128, NPR], BF16, tag=f"ms_{br}", name=f"ms_{br}{c}")
        nc.scalar.copy(ms[:], mp[:])
        prod = wpool.tile([128, NPR], BF16, tag=f"prod_{br}",
                          name=f"prod_{br}{c}")
        nc.vector.tensor_mul(prod[:], ms[:], ctiles[c][:])
        nc.vector.reduce_sum(
            out=mv_out,
            in_=prod[:].rearrange("p (a k) -> p a k", k=R),
            axis=AX.X,
        )

    mvt = [cpool.tile([128, MH], F32, tag=f"mvt{c}", name=f"mvt{c}")
           for c in range(NB)]
    logits = cpool.tile([128, NB], F32, tag="logits")

    def do_c(c):
        mvc = wpool.tile([128, MH], F32, tag="mvc", name=f"mvc{c}")
        branch_mv("c", c, wrg, emb_c, mvc[:])
        junk = lpool.tile([128, MH], F32, tag="junk", name=f"junk{c}")
        nc.vector.tensor_mul(junk[:], mvt[c][:], mvc[:])
        nc.vector.reduce_sum(out=logits[:, c:c + 1], in_=junk[:], axis=AX.X)

    # per 4-chunk group (matching the gather pieces): t chunks, then c
    for g in range(4):
        for c in range(4 * g, 4 * g + 4):
            branch_mv("t", c, wrt, emb_t, mvt[c][:])
        for c in range(4 * g, 4 * g + 4):
            do_c(c)

    # ---- batched loss tail: softplus(l) - l*y over [128, NB] -----------
    ab = lpool.tile([128, NB], F32, tag="ab")
    nc.scalar.activation(ab[:], logits[:], AF.Abs)
    ex = lpool.tile([128, NB], F32, tag="ex")
    nc.scalar.activation(ex[:], ab[:], AF.Exp, scale=-1.0)
    l1p = lpool.tile([128, NB], F32, tag="l1p")
    nc.scalar.activation(l1p[:], ex[:], AF.Ln, bias=1.0)
    rl = lpool.tile([128, NB], F32, tag="rl")
    nc.scalar.activation(rl[:], logits[:], AF.Relu)
    sp = lpool.tile([128, NB], F32, tag="sp")
    nc.vector.tensor_add(sp[:], rl[:], l1p[:])
    ll = lpool.tile([128, NB], F32, tag="ll")
    nc.vector.tensor_mul(ll[:], logits[:], labels[:])
    dvec = lpool.tile([128, NB], F32, tag="dvec")
    nc.vector.tensor_sub(dvec[:], sp[:], ll[:])

    srow = cpool.tile([128, 1], F32, tag="srow")
    nc.vector.reduce_sum(out=srow[:], in_=dvec[:], axis=AX.X)
    fin = ptv.tile([1, 1], F32, tag="ptv", name="pfin")
    nc.tensor.matmul(fin[:], srow[:], ones128[:], start=True, stop=True)
    res = cpool.tile([1, 1], F32, tag="res")
    nc.scalar.copy(res[:], fin[:])
    nc.sync.dma_start(out=io["out"][:, :], in_=res[:])


_PROGRAM = None


def _get_program():
    global _PROGRAM
    if _PROGRAM is not None:
        return _PROGRAM
    nc = bacc.Bacc("TRN2", target_bir_lowering=False, debug=False,
                   num_devices=N_CORES, num_swdge_queues=4)
    io = {
        "ttab": nc.dram_tensor("ttab", [W, EPAD], BF16, kind="ExternalInput").ap(),
        "wrt": nc.dram_tensor("wrt", [EPAD, NPR], BF16, kind="ExternalInput").ap(),
        "wrg": nc.dram_tensor("wrg", [EPAD, NPR], BF16, kind="ExternalInput").ap(),
        "vtile": nc.dram_tensor("vtile", [MH, NPR], BF16, kind="ExternalInput").ap(),
        "h2k": nc.dram_tensor("h2k", [H, H], BF16, kind="ExternalInput").ap(),
        "h2b": nc.dram_tensor("h2b", [H, 1], F32, kind="ExternalInput").ap(),
        "h1k": nc.dram_tensor("h1k", [H, 1], F32, kind="ExternalInput").ap(),
        "h1b": nc.dram_tensor("h1b", [H, 1], F32, kind="ExternalInput").ap(),
        "identb": nc.dram_tensor("identb", [128, 128], BF16, kind="ExternalInput").ap(),
        "times": nc.dram_tensor("times", [1, BC], BF16, kind="ExternalInput").ap(),
        "labels": nc.dram_tensor("labels", [128, NB], F32, kind="ExternalInput").ap(),
        "idx_t": nc.dram_tensor("idx_t", [128, BC // 16], I16, kind="ExternalInput").ap(),
        "idx_c": nc.dram_tensor("idx_c", [128, BC // 16], I16, kind="ExternalInput").ap(),
        "out": nc.dram_tensor("out", [1, 1], F32, kind="ExternalOutput").ap(),
    }
    for s in range(4):
        io[f"ctab{s}"] = nc.dram_tensor(f"ctab{s}", [W, EPAD], BF16,
                                        kind="ExternalInput").ap()
    with tile.TileContext(nc) as tc:
        with ExitStack() as ctx:
            _build_kernel(ctx, tc, io)
    nc.compile()
    _PROGRAM = nc
    return nc


def _pad_table(tab):
    out = np.zeros((V, EPAD), dtype=ml_dtypes.bfloat16)
    out[:, :EMB] = np.asarray(tab).astype(ml_dtypes.bfloat16)
    out[:, EMB] = 1.0
    return out


def _precompute_weights(h1_k, h1_b, h2_k, h2_b, evoke_k, evoke_b,
                        last_k, last_b):
    """Affine rank-R tv basis + folded contraction weights (float64)."""
    h1_k = np.asarray(h1_k, np.float64)
    h1_b = np.asarray(h1_b, np.float64)
    h2_k = np.asarray(h2_k, np.float64)
    h2_b = np.asarray(h2_b, np.float64)
    g = np.linspace(0.0, 1.0, 8193, dtype=np.float64).reshape(-1, 1)
    h1g = np.tanh(g @ h1_k.reshape(1, H) + h1_b.reshape(H))
    tvg = np.tanh(h1g @ h2_k + h2_b.reshape(H))
    m = tvg.mean(0)
    _, _, vt = np.linalg.svd(tvg - m, full_matrices=False)
    v3 = vt[:R - 1].T                                   # [100, R-1]
    b_aff = np.concatenate([m.reshape(-1, 1), v3], 1)   # [100, R]
    vaff_h = np.zeros((MH, R))
    vaff_h[:H, 1:] = v3
    vaff_h[H, 0] = 1.0
    vaff_h[H, 1:] = -(m @ v3)

    evoke_pad = np.zeros((EPAD, H * H))
    evoke_pad[:EMB] = np.asarray(evoke_k, np.float64)
    evoke_pad[EMB] = np.asarray(evoke_b, np.float64)
    wr = (evoke_pad.reshape(EPAD * H, H) @ b_aff).reshape(EPAD, H, R)
    wr_full = np.zeros((EPAD, MH, R))
    wr_full[:, :H, :] = wr
    wr_full[EMB, H, 0] = 1.0
    lastkh = np.vstack([np.asarray(last_k, np.float64),
                        np.asarray(last_b, np.float64).reshape(1, EMB)])
    gh = lastkh @ lastkh.T
    wrgh = np.einsum('epk,pq->eqk', wr_full, gh)

    wrt = wr_full.reshape(EPAD, NPR).astype(ml_dtypes.bfloat16)
    wrg = wrgh.reshape(EPAD, NPR).astype(ml_dtypes.bfloat16)
    vtile = np.tile(vaff_h, (1, MH)).astype(ml_dtypes.bfloat16)
    h2kc = h2_k.astype(ml_dtypes.bfloat16)
    h2bc = h2_b.reshape(H, 1).astype(np.float32).copy()
    h1kc = h1_k.reshape(1, H).T.astype(np.float32).copy()
    h1bc = h1_b.reshape(H, 1).astype(np.float32).copy()
    return wrt, wrg, vtile, h2kc, h2bc, h1kc, h1bc


def build_in_maps(targets, contexts, times, labels, targetemb, contextemb,
                  h1_k, h1_b, h2_k, h2_b, evoke_k, evoke_b, last_k, last_b):
    global LAST_PERMS
    ttab = _pad_table(targetemb)
    ctab = _pad_table(contextemb)
    wrt, wrg, vtile, h2kc, h2bc, h1kc, h1bc = _precompute_weights(
        h1_k, h1_b, h2_k, h2_b, evoke_k, evoke_b, last_k, last_b)
    identb = np.eye(128, dtype=ml_dtypes.bfloat16)
    targets = np.asarray(targets).astype(np.int64)
    contexts = np.asarray(contexts).astype(np.int64)
    times = np.asarray(times).astype(np.float32)
    labels = np.asarray(labels).astype(np.float32)

    order_t = np.argsort(targets, kind="stable")
    in_maps = []
    perms = []
    for k in range(N_CORES):
        i_k = order_t[k * BC:(k + 1) * BC]
        j_k = i_k[np.argsort(contexts[i_k], kind="stable")]
        perms.append(j_k)
        tv = targets[j_k]
        cv = contexts[j_k]
        off_t = min(int(tv.min()), V - W)
        t_loc = tv - off_t
        assert t_loc.min() >= 0 and t_loc.max() < W, "t window overflow"
        m = {
            "ttab": ttab[off_t:off_t + W],
            "wrt": wrt, "wrg": wrg, "vtile": vtile,
            "h2k": h2kc, "h2b": h2bc,
            "h1k": h1kc, "h1b": h1bc, "identb": identb,
            "times": times[j_k].astype(ml_dtypes.bfloat16).reshape(1, BC),
            "labels": labels[j_k].reshape(NB, 128).T.copy(),
            "idx_t": _wrap16(t_loc),
        }
        c_loc = np.empty(BC, dtype=np.int64)
        for s in range(4):
            seg = cv[SEG * s:SEG * (s + 1)]
            base = min(int(seg[0]), V - W)
            loc = seg - base
            assert loc.min() >= 0 and loc.max() < W, "c window overflow"
            c_loc[SEG * s:SEG * (s + 1)] = loc
            m[f"ctab{s}"] = ctab[base:base + W]
        m["idx_c"] = _wrap16(c_loc)
        in_maps.append(m)
    LAST_PERMS = perms
    return in_maps


def kernel(**inputs) -> np.ndarray:
    nc = _get_program()
    in_maps = build_in_maps(**inputs)
    r = run_bass_kernel_spmd(nc, in_maps, list(range(N_CORES)))
    total = np.float64(0.0)
    for m in r.results:
        total += np.float64(m["out"][0, 0])
    return np.float32(total / B)


# revision 12
# speedup vs baseline: 3.1080x; 1.0052x over previous
"""Trainium2 Bass kernel for nn_DiffTime (embedding_lookup, 8 NeuronCores).

Reference computation:
    h1 = tanh(times * h1_k + h1_b)            [B, 100]
    tv = tanh(h1 @ h2_k + h2_b)               [B, 100]
    mat_x = (emb_x @ evoke_k + evoke_b)       [B, 100p, 100h]   (x in {target, context})
    mv_x = einsum('bph,bh->bp', mat_x, tv)    [B, 100]
    vect_x = mv_x @ last_k + last_b           [B, 300]
    logits = sum(vect_t * vect_c, -1)         [B]
    out = mean(softplus(logits) - logits * labels)

Kernel strategy (data-parallel, 2048 items/core, no collectives):

* tv rows lie on a smooth 1-D curve in R^100; an affine rank-4 basis
  (mean + 3 SVD directions of the centered curve, c0 == 1 by a
  homogeneous-coordinate trick) reproduces the final loss to ~4e-6.
  The h-contraction is folded into the weights on the host:
  Wr[e,(p,k)] = sum_h evoke_pad[e,p*100+h]*B_aff[h,k], so the kernel
  contracts emb (384-padded, homogeneous col 300 == 1) against a
  [384, 404] matrix and reduces over k=4 with a broadcast coefficient
  tile.  The Gram matrix Gh = lastkh @ lastkh.T (which turns the two
  [B,300] branch vectors into a [101]x[101] bilinear form) is folded
  into the context branch weights as well, so logits are a single
  fused multiply-reduce of the two [128,101] mv tiles.

* Gathers are single-stage on both branches (no scratch / realign):
  - batch items are assigned to cores by a global argsort of targets,
    so each core's target rows fall inside one 32768-row table window
    (span ~12.5k) => one 2048-row int16 dma_gather from a per-core
    window slice fed as input;
  - within each core, items are processed in context-sorted order
    (the loss is an order-invariant mean, so any processing order
    works as long as times/labels/indices are permuted consistently);
    the sorted contexts are cut at ranks 512/1024/1536 and gathered
    with four 512-row dma_gathers from per-core percentile windows
    (span of 512 sorted uniform draws ~26k < 32768).

* emb transposes ([b,e] -> [e,b] for the PE contraction) use the XBAR
  dma_start_transpose (SBUF->SBUF, [128,384] -> [128,3,128]) on the
  otherwise-idle SP queue instead of PE transposes + PSUM evictions.
"""

import sys

for _p in ("/opt/trn_rl_repo", "/opt/trn_rl_repo/concourse"):
    if _p not in sys.path:
        sys.path.insert(0, _p)

from contextlib import ExitStack

import ml_dtypes
import numpy as np

import concourse.bacc as bacc
import concourse.bass as bass
import concourse.tile as tile
from concourse import mybir
from concourse.bass_utils import run_bass_kernel_spmd

F32 = mybir.dt.float32
BF16 = mybir.dt.bfloat16
I16 = mybir.dt.int16
AF = mybir.ActivationFunctionType
AX = mybir.AxisListType
OP = mybir.AluOpType

N_CORES = 8
B = 16384
BC = B // N_CORES          # 2048 batch items per core
NB = BC // 128             # 16 chunks of 128 batch rows
V = 100000
EMB = 300
EPAD = 384                 # padded embedding row (col 300 = 1.0, rest 0)
H = 100
MH = H + 1                 # homogeneous mv size
R = 4                      # affine tv-basis rank (c0 == 1)
NPR = MH * R               # 404 contracted columns
W = 32768                  # per-core table window (int16-addressable)
SEG = 512                  # context gather piece (4 x 512 = 2048)

USE_DMA_TRANSPOSE = True

LAST_PERMS = None          # debug: per-core batch permutation of last build


def _wrap16(v):
    """int16 index array -> dma_gather SBUF layout [128, len//16]."""
    v = np.asarray(v, dtype=np.int16)
    a = v.reshape(-1, 16).T
    return np.tile(a, (8, 1))


def _build_kernel(ctx: ExitStack, tc: "tile.TileContext", io: dict):
    nc = tc.nc

    cpool = ctx.enter_context(tc.tile_pool(name="const", bufs=1))
    wpool = ctx.enter_context(tc.tile_pool(name="work", bufs=4))
    tvpool = ctx.enter_context(tc.tile_pool(name="tvp", bufs=3))
    lpool = ctx.enter_context(tc.tile_pool(name="loss", bufs=2))
    pmm = ctx.enter_context(tc.tile_pool(name="pmm", bufs=3, space="PSUM"))
    ptv = ctx.enter_context(tc.tile_pool(name="ptv", bufs=2, space="PSUM"))
    if not USE_DMA_TRANSPOSE:
        ptr = ctx.enter_context(tc.tile_pool(name="ptr", bufs=3, space="PSUM"))

    # ---- small resident constants (SP queue: these come first so the
    # gathers, which only need the index tiles, start immediately) ------
    idx_t = cpool.tile([128, BC // 16], I16, tag="idx_t")
    nc.sync.dma_start(out=idx_t[:], in_=io["idx_t"][:, :])
    idx_c = cpool.tile([128, BC // 16], I16, tag="idx_c")
    nc.sync.dma_start(out=idx_c[:], in_=io["idx_c"][:, :])

    # ---- gathers: one per branch-piece, Q7 queue order = emission -----
    emb_t = [cpool.tile([128, 4, EPAD], BF16, tag=f"emb_t{g}", name=f"emb_t{g}")
             for g in range(4)]
    emb_c = [cpool.tile([128, 4, EPAD], BF16, tag=f"emb_c{g}", name=f"emb_c{g}")
             for g in range(4)]
    qn = 0
    for s in range(4):
        nc.gpsimd.dma_gather(
            emb_t[s][:], io["ttab"][:, :],
            idx_t[:, (SEG // 16) * s:(SEG // 16) * (s + 1)], SEG, SEG, EPAD,
            queue_num=qn % 4, single_packet=False,
        )
        qn += 1
        nc.gpsimd.dma_gather(
            emb_c[s][:], io[f"ctab{s}"][:, :],
            idx_c[:, (SEG // 16) * s:(SEG // 16) * (s + 1)], SEG, SEG, EPAD,
            queue_num=qn % 4, single_packet=False,
        )
        qn += 1

    # ---- remaining constants: small ones + big weights, split SP/Act ----
    times = cpool.tile([1, BC], BF16, tag="times")
    nc.sync.dma_start(out=times[:], in_=io["times"][:, :])
    h1k = cpool.tile([H, 1], F32, tag="h1k")
    nc.scalar.dma_start(out=h1k[:], in_=io["h1k"][:, :])
    h1b = cpool.tile([H, 1], F32, tag="h1b")
    nc.scalar.dma_start(out=h1b[:], in_=io["h1b"][:, :])
    h2k = cpool.tile([H, H], BF16, tag="h2k")
    nc.sync.dma_start(out=h2k[:], in_=io["h2k"][:, :])
    h2b = cpool.tile([H, 1], F32, tag="h2b")
    nc.scalar.dma_start(out=h2b[:], in_=io["h2b"][:, :])
    vtile = cpool.tile([MH, NPR], BF16, tag="vtile")
    nc.sync.dma_start(out=vtile[:], in_=io["vtile"][:, :])
    labels = cpool.tile([128, NB], F32, tag="labels")
    nc.scalar.dma_start(out=labels[:], in_=io["labels"][:, :])
    identb = cpool.tile([128, 128], BF16, tag="identb")
    nc.scalar.dma_start(out=identb[:], in_=io["identb"][:, :])
    wrt = [cpool.tile([128, NPR], BF16, tag=f"wrt{j}", name=f"wrt{j}")
           for j in range(3)]
    wrg = [cpool.tile([128, NPR], BF16, tag=f"wrg{j}", name=f"wrg{j}")
           for j in range(3)]
    for j in range(3):
        nc.sync.dma_start(out=wrt[j][:], in_=io["wrt"][128 * j:128 * (j + 1), :])
    for j in range(3):
        nc.scalar.dma_start(out=wrg[j][:], in_=io["wrg"][128 * j:128 * (j + 1), :])


    ones1 = cpool.tile([1, H], BF16, tag="ones1")
    nc.vector.memset(ones1[:], 1.0)
    ones128 = cpool.tile([128, 1], F32, tag="ones128")
    nc.vector.memset(ones128[:], 1.0)

    # ---- time MLP -> broadcast coefficient tiles ctile[c] --------------
    tvh_bufs = [cpool.tile([MH, 128], BF16, tag=f"tvhb{i}", name=f"tvhb{i}")
                for i in range(3)]
    for i in range(3):
        nc.vector.memset(tvh_bufs[i][:], 1.0)
    ctiles = []
    for c in range(NB):
        bcast = ptv.tile([H, 128], F32, tag="ptv", name=f"bcast{c}")
        nc.tensor.matmul(bcast[:], ones1[:], times[0:1, 128 * c:128 * (c + 1)],
                         start=True, stop=True)
        h1T = tvpool.tile([H, 128], BF16, tag="h1T")
        nc.scalar.activation(h1T[:], bcast[:], AF.Tanh, bias=h1b[:],
                             scale=h1k[:])
        tvp = ptv.tile([H, 128], F32, tag="ptv", name=f"tvp{c}")
        nc.tensor.matmul(tvp[:], h2k[:], h1T[:], start=True, stop=True)
        tvhT = tvh_bufs[c % 3]
        nc.scalar.activation(tvhT[0:H, :], tvp[:], AF.Tanh, bias=h2b[:])
        cwp = pmm.tile([128, NPR], F32, tag="mp", name=f"cwp{c}")
        nc.tensor.matmul(cwp[:], tvhT[:], vtile[:], start=True, stop=True)
        ct = cpool.tile([128, NPR], BF16, tag=f"ct{c}", name=f"ct{c}")
        nc.vector.tensor_copy(ct[:], cwp[:])
        ctiles.append(ct)

    # ---- per-chunk branch contraction ---------------------------------
    et12 = {}

    def group_transpose(br, g, emb):
        t = wpool.tile([128, 12, 128], BF16, tag=f"et12_{br}",
                       name=f"et12_{br}{g}")
        nc.sync.dma_start_transpose(
            t[:], emb[g][:].rearrange("p c e -> p (c e)"))
        et12[br, g] = t

    def branch_mv(br, c, wr, emb, mv_out):
        if USE_DMA_TRANSPOSE:
            if (br, c // 4) not in et12:
                group_transpose(br, c // 4, emb)
            t = et12[br, c // 4]
            lhs = [t[:, (c % 4) * 3 + j, :] for j in range(3)]
        else:
            lhs = []
            for j in range(3):
                tpp = ptr.tile([128, 128], BF16, tag="pt", name=f"pt{br}{c}{j}")
                nc.tensor.transpose(
                    tpp[:], emb[c // 4][:, c % 4, 128 * j:128 * (j + 1)],
                    identb[:])
                et = wpool.tile([128, 128], BF16, tag=f"et{j}_{br}",
                                name=f"et{j}_{br}{c}")
                nc.vector.tensor_copy(et[:], tpp[:])
                lhs.append(et[:])
        mp = pmm.tile([128, NPR], F32, tag="mp", name=f"mp_{br}{c}")
        for j in range(3):
            nc.tensor.matmul(mp[:], lhs[j], wr[j][:], start=(j == 0),
                             stop=(j == 2))
        ms = wpool.tile([128, NPR], BF16, tag=f"ms_{br}", name=f"ms_{br}{c}")
        nc.scalar.copy(ms[:], mp[:])
        prod = wpool.tile([128, NPR], BF16, tag=f"prod_{br}",
                          name=f"prod_{br}{c}")
        nc.vector.tensor_mul(prod[:], ms[:], ctiles[c][:])
        nc.vector.reduce_sum(
            out=mv_out,
            in_=prod[:].rearrange("p (a k) -> p a k", k=R),
            axis=AX.X,
        )

    mvt = [cpool.tile([128, MH], F32, tag=f"mvt{c}", name=f"mvt{c}")
           for c in range(NB)]
    logits = cpool.tile([128, NB], F32, tag="logits")

    def do_c(c):
        mvc = wpool.tile([128, MH], F32, tag="mvc", name=f"mvc{c}")
        branch_mv("c", c, wrg, emb_c, mvc[:])
        junk = lpool.tile([128, MH], F32, tag="junk", name=f"junk{c}")
        nc.vector.tensor_mul(junk[:], mvt[c][:], mvc[:])
        nc.vector.reduce_sum(out=logits[:, c:c + 1], in_=junk[:], axis=AX.X)

    # per 4-chunk group (matching the gather pieces): t chunks, then c
    for g in range(4):
        for c in range(4 * g, 4 * g + 4):
            branch_mv("t", c, wrt, emb_t, mvt[c][:])
        for c in range(4 * g, 4 * g + 4):
            do_c(c)

    # ---- batched loss tail: softplus(l) - l*y over [128, NB] -----------
    ab = lpool.tile([128, NB], F32, tag="ab")
    nc.scalar.activation(ab[:], logits[:], AF.Abs)
    ex = lpool.tile([128, NB], F32, tag="ex")
    nc.scalar.activation(ex[:], ab[:], AF.Exp, scale=-1.0)
    l1p = lpool.tile([128, NB], F32, tag="l1p")
    nc.scalar.activation(l1p[:], ex[:], AF.Ln, bias=1.0)
    rl = lpool.tile([128, NB], F32, tag="rl")
    nc.scalar.activation(rl[:], logits[:], AF.Relu)
    sp = lpool.tile([128, NB], F32, tag="sp")
    nc.vector.tensor_add(sp[:], rl[:], l1p[:])
    ll = lpool.tile([128, NB], F32, tag="ll")
    nc.vector.tensor_mul(ll[:], logits[:], labels[:])
    dvec = lpool.tile([128, NB], F32, tag="dvec")
    nc.vector.tensor_sub(dvec[:], sp[:], ll[:])

    srow = cpool.tile([128, 1], F32, tag="srow")
    nc.vector.reduce_sum(out=srow[:], in_=dvec[:], axis=AX.X)
    fin = ptv.tile([1, 1], F32, tag="ptv", name="pfin")
    nc.tensor.matmul(fin[:], srow[:], ones128[:], start=True, stop=True)
    res = cpool.tile([1, 1], F32, tag="res")
    nc.scalar.copy(res[:], fin[:])
    nc.sync.dma_start(out=io["out"][:, :], in_=res[:])


_PROGRAM = None


def _get_program():
    global _PROGRAM
    if _PROGRAM is not None:
        return _PROGRAM
    nc = bacc.Bacc("TRN2", target_bir_lowering=False, debug=False,
                   num_devices=N_CORES, num_swdge_queues=4)
    io = {
        "ttab": nc.dram_tensor("ttab", [W, EPAD], BF16, kind="ExternalInput").ap(),
        "wrt": nc.dram_tensor("wrt", [EPAD, NPR], BF16, kind="ExternalInput").ap(),
        "wrg": nc.dram_tensor("wrg", [EPAD, NPR], BF16, kind="ExternalInput").ap(),
        "vtile": nc.dram_tensor("vtile", [MH, NPR], BF16, kind="ExternalInput").ap(),
        "h2k": nc.dram_tensor("h2k", [H, H], BF16, kind="ExternalInput").ap(),
        "h2b": nc.dram_tensor("h2b", [H, 1], F32, kind="ExternalInput").ap(),
        "h1k": nc.dram_tensor("h1k", [H, 1], F32, kind="ExternalInput").ap(),
        "h1b": nc.dram_tensor("h1b", [H, 1], F32, kind="ExternalInput").ap(),
        "identb": nc.dram_tensor("identb", [128, 128], BF16, kind="ExternalInput").ap(),
        "times": nc.dram_tensor("times", [1, BC], BF16, kind="ExternalInput").ap(),
        "labels": nc.dram_tensor("labels", [128, NB], F32, kind="ExternalInput").ap(),
        "idx_t": nc.dram_tensor("idx_t", [128, BC // 16], I16, kind="ExternalInput").ap(),
        "idx_c": nc.dram_tensor("idx_c", [128, BC // 16], I16, kind="ExternalInput").ap(),
        "out": nc.dram_tensor("out", [1, 1], F32, kind="ExternalOutput").ap(),
    }
    for s in range(4):
        io[f"ctab{s}"] = nc.dram_tensor(f"ctab{s}", [W, EPAD], BF16,
                                        kind="ExternalInput").ap()
    with tile.TileContext(nc) as tc:
        with ExitStack() as ctx:
            _build_kernel(ctx, tc, io)
    nc.compile()
    _PROGRAM = nc
    return nc


def _pad_table(tab):
    out = np.zeros((V, EPAD), dtype=ml_dtypes.bfloat16)
    out[:, :EMB] = np.asarray(tab).astype(ml_dtypes.bfloat16)
    out[:, EMB] = 1.0
    return out


def _precompute_weights(h1_k, h1_b, h2_k, h2_b, evoke_k, evoke_b,
                        last_k, last_b):
    """Affine rank-R tv basis + folded contraction weights (float64)."""
    h1_k = np.asarray(h1_k, np.float64)
    h1_b = np.asarray(h1_b, np.float64)
    h2_k = np.asarray(h2_k, np.float64)
    h2_b = np.asarray(h2_b, np.float64)
    g = np.linspace(0.0, 1.0, 8193, dtype=np.float64).reshape(-1, 1)
    h1g = np.tanh(g @ h1_k.reshape(1, H) + h1_b.reshape(H))
    tvg = np.tanh(h1g @ h2_k + h2_b.reshape(H))
    m = tvg.mean(0)
    _, _, vt = np.linalg.svd(tvg - m, full_matrices=False)
    v3 = vt[:R - 1].T                                   # [100, R-1]
    b_aff = np.concatenate([m.reshape(-1, 1), v3], 1)   # [100, R]
    vaff_h = np.zeros((MH, R))
    vaff_h[:H, 1:] = v3
    vaff_h[H, 0] = 1.0
    vaff_h[H, 1:] = -(m @ v3)

    evoke_pad = np.zeros((EPAD, H * H))
    evoke_pad[:EMB] = np.asarray(evoke_k, np.float64)
    evoke_pad[EMB] = np.asarray(evoke_b, np.float64)
    wr = (evoke_pad.reshape(EPAD * H, H) @ b_aff).reshape(EPAD, H, R)
    wr_full = np.zeros((EPAD, MH, R))
    wr_full[:, :H, :] = wr
    wr_full[EMB, H, 0] = 1.0
    lastkh = np.vstack([np.asarray(last_k, np.float64),
                        np.asarray(last_b, np.float64).reshape(1, EMB)])
    gh = lastkh @ lastkh.T
    wrgh = np.einsum('epk,pq->eqk', wr_full, gh)

    wrt = wr_full.reshape(EPAD, NPR).astype(ml_dtypes.bfloat16)
    wrg = wrgh.reshape(EPAD, NPR).astype(ml_dtypes.bfloat16)
    vtile = np.tile(vaff_h, (1, MH)).astype(ml_dtypes.bfloat16)
    h2kc = h2_k.astype(ml_dtypes.bfloat16)
    h2bc = h2_b.reshape(H, 1).astype(np.float32).copy()
    h1kc = h1_k.reshape(1, H).T.astype(np.float32).copy()
    h1bc = h1_b.reshape(H, 1).astype(np.float32).copy()
    return wrt, wrg, vtile, h2kc, h2bc, h1kc, h1bc


def build_in_maps(targets, contexts, times, labels, targetemb, contextemb,
                  h1_k, h1_b, h2_k, h2_b, evoke_k, evoke_b, last_k, last_b):
    global LAST_PERMS
    ttab = _pad_table(targetemb)
    ctab = _pad_table(contextemb)
    wrt, wrg, vtile, h2kc, h2bc, h1kc, h1bc = _precompute_weights(
        h1_k, h1_b, h2_k, h2_b, evoke_k, evoke_b, last_k, last_b)
    identb = np.eye(128, dtype=ml_dtypes.bfloat16)
    targets = np.asarray(targets).astype(np.int64)
    contexts = np.asarray(contexts).astype(np.int64)
    times = np.asarray(times).astype(np.float32)
    labels = np.asarray(labels).astype(np.float32)

    order_t = np.argsort(targets, kind="stable")
    in_maps = []
    perms = []
    for k in range(N_CORES):
        i_k = order_t[k * BC:(k + 1) * BC]
        j_k = i_k[np.argsort(contexts[i_k], kind="stable")]
        perms.append(j_k)
        tv = targets[j_k]
        cv = contexts[j_k]
        off_t = min(int(tv.min()), V - W)
        t_loc = tv - off_t
        assert t_loc.min() >= 0 and t_loc.max() < W, "t window overflow"
        m = {
            "ttab": ttab[off_t:off_t + W],
            "wrt": wrt, "wrg": wrg, "vtile": vtile,
            "h2k": h2kc, "h2b": h2bc,
            "h1k": h1kc, "h1b": h1bc, "identb": identb,
            "times": times[j_k].astype(ml_dtypes.bfloat16).reshape(1, BC),
            "labels": labels[j_k].reshape(NB, 128).T.copy(),
            "idx_t": _wrap16(t_loc),
        }
        c_loc = np.empty(BC, dtype=np.int64)
        for s in range(4):
            seg = cv[SEG * s:SEG * (s + 1)]
            base = min(int(seg[0]), V - W)
            loc = seg - base
            assert loc.min() >= 0 and loc.max() < W, "c window overflow"
            c_loc[SEG * s:SEG * (s + 1)] = loc
            m[f"ctab{s}"] = ctab[base:base + W]
        m["idx_c"] = _wrap16(c_loc)
        in_maps.append(m)
    LAST_PERMS = perms
    return in_maps


def kernel(**inputs) -> np.ndarray:
    nc = _get_program()
    in_maps = build_in_maps(**inputs)
    r = run_bass_kernel_spmd(nc, in_maps, list(range(N_CORES)))
    total = np.float64(0.0)
    for m in r.results:
        total += np.float64(m["out"][0, 0])
    return np.float32(total / B)


# revision 13
# speedup vs baseline: 3.1157x; 1.0025x over previous
"""Trainium2 Bass kernel for nn_DiffTime (embedding_lookup, 8 NeuronCores).

Reference computation:
    h1 = tanh(times * h1_k + h1_b)            [B, 100]
    tv = tanh(h1 @ h2_k + h2_b)               [B, 100]
    mat_x = (emb_x @ evoke_k + evoke_b)       [B, 100p, 100h]   (x in {target, context})
    mv_x = einsum('bph,bh->bp', mat_x, tv)    [B, 100]
    vect_x = mv_x @ last_k + last_b           [B, 300]
    logits = sum(vect_t * vect_c, -1)         [B]
    out = mean(softplus(logits) - logits * labels)

Kernel strategy (data-parallel, 2048 items/core, no collectives):

* tv rows lie on a smooth 1-D curve in R^100; an affine rank-4 basis
  (mean + 3 SVD directions of the centered curve, c0 == 1 by a
  homogeneous-coordinate trick) reproduces the final loss to ~4e-6.
  The h-contraction is folded into the weights on the host:
  Wr[e,(p,k)] = sum_h evoke_pad[e,p*100+h]*B_aff[h,k], so the kernel
  contracts emb (384-padded, homogeneous col 300 == 1) against a
  [384, 404] matrix and reduces over k=4 with a broadcast coefficient
  tile.  The Gram matrix Gh = lastkh @ lastkh.T (which turns the two
  [B,300] branch vectors into a [101]x[101] bilinear form) is folded
  into the context branch weights as well, so logits are a single
  fused multiply-reduce of the two [128,101] mv tiles.

* Gathers are single-stage on both branches (no scratch / realign):
  - batch items are assigned to cores by a global argsort of targets,
    so each core's target rows fall inside one 32768-row table window
    (span ~12.5k) => one 2048-row int16 dma_gather from a per-core
    window slice fed as input;
  - within each core, items are processed in context-sorted order
    (the loss is an order-invariant mean, so any processing order
    works as long as times/labels/indices are permuted consistently);
    the sorted contexts are cut at ranks 512/1024/1536 and gathered
    with four 512-row dma_gathers from per-core percentile windows
    (span of 512 sorted uniform draws ~26k < 32768).

* emb transposes ([b,e] -> [e,b] for the PE contraction) use the XBAR
  dma_start_transpose (SBUF->SBUF, [128,384] -> [128,3,128]) on the
  otherwise-idle SP queue instead of PE transposes + PSUM evictions.
"""

import sys

for _p in ("/opt/trn_rl_repo", "/opt/trn_rl_repo/concourse"):
    if _p not in sys.path:
        sys.path.insert(0, _p)

from contextlib import ExitStack

import ml_dtypes
import numpy as np

import concourse.bacc as bacc
import concourse.bass as bass
import concourse.tile as tile
from concourse import mybir
from concourse.bass_utils import run_bass_kernel_spmd

F32 = mybir.dt.float32
BF16 = mybir.dt.bfloat16
I16 = mybir.dt.int16
AF = mybir.ActivationFunctionType
AX = mybir.AxisListType
OP = mybir.AluOpType

N_CORES = 8
B = 16384
BC = B // N_CORES          # 2048 batch items per core
NB = BC // 128             # 16 chunks of 128 batch rows
V = 100000
EMB = 300
EPAD = 384                 # padded embedding row (col 300 = 1.0, rest 0)
H = 100
MH = H + 1                 # homogeneous mv size
R = 4                      # affine tv-basis rank (c0 == 1)
NPR = MH * R               # 404 contracted columns
W = 32768                  # per-core table window (int16-addressable)
SEG = 512                  # context gather piece (4 x 512 = 2048)

USE_DMA_TRANSPOSE = True

LAST_PERMS = None          # debug: per-core batch permutation of last build


def _wrap16(v):
    """int16 index array -> dma_gather SBUF layout [128, len//16]."""
    v = np.asarray(v, dtype=np.int16)
    a = v.reshape(-1, 16).T
    return np.tile(a, (8, 1))


def _build_kernel(ctx: ExitStack, tc: "tile.TileContext", io: dict):
    nc = tc.nc

    cpool = ctx.enter_context(tc.tile_pool(name="const", bufs=1))
    wpool = ctx.enter_context(tc.tile_pool(name="work", bufs=4))
    tvpool = ctx.enter_context(tc.tile_pool(name="tvp", bufs=3))
    lpool = ctx.enter_context(tc.tile_pool(name="loss", bufs=2))
    pmm = ctx.enter_context(tc.tile_pool(name="pmm", bufs=3, space="PSUM"))
    ptv = ctx.enter_context(tc.tile_pool(name="ptv", bufs=2, space="PSUM"))
    if not USE_DMA_TRANSPOSE:
        ptr = ctx.enter_context(tc.tile_pool(name="ptr", bufs=3, space="PSUM"))

    # ---- small resident constants (SP queue: these come first so the
    # gathers, which only need the index tiles, start immediately) ------
    idx_t = cpool.tile([128, BC // 16], I16, tag="idx_t")
    nc.sync.dma_start(out=idx_t[:], in_=io["idx_t"][:, :])
    idx_c = cpool.tile([128, BC // 16], I16, tag="idx_c")
    nc.sync.dma_start(out=idx_c[:], in_=io["idx_c"][:, :])

    # ---- gathers: one per branch-piece, Q7 queue order = emission -----
    emb_t = [cpool.tile([128, 4, EPAD], BF16, tag=f"emb_t{g}", name=f"emb_t{g}")
             for g in range(4)]
    emb_c = [cpool.tile([128, 4, EPAD], BF16, tag=f"emb_c{g}", name=f"emb_c{g}")
             for g in range(4)]
    qn = 0
    for s in range(4):
        nc.gpsimd.dma_gather(
            emb_t[s][:], io["ttab"][:, :],
            idx_t[:, (SEG // 16) * s:(SEG // 16) * (s + 1)], SEG, SEG, EPAD,
            queue_num=qn % 4, single_packet=False,
        )
        qn += 1
        nc.gpsimd.dma_gather(
            emb_c[s][:], io[f"ctab{s}"][:, :],
            idx_c[:, (SEG // 16) * s:(SEG // 16) * (s + 1)], SEG, SEG, EPAD,
            queue_num=qn % 4, single_packet=False,
        )
        qn += 1

    # ---- remaining constants: small ones + big weights, split SP/Act ----
    times = cpool.tile([1, BC], BF16, tag="times")
    nc.sync.dma_start(out=times[:], in_=io["times"][:, :])
    h1k = cpool.tile([H, 1], F32, tag="h1k")
    nc.scalar.dma_start(out=h1k[:], in_=io["h1k"][:, :])
    h1b = cpool.tile([H, 1], F32, tag="h1b")
    nc.scalar.dma_start(out=h1b[:], in_=io["h1b"][:, :])
    h2k = cpool.tile([H, H], BF16, tag="h2k")
    nc.sync.dma_start(out=h2k[:], in_=io["h2k"][:, :])
    h2b = cpool.tile([H, 1], F32, tag="h2b")
    nc.scalar.dma_start(out=h2b[:], in_=io["h2b"][:, :])
    vtile = cpool.tile([MH, NPR], BF16, tag="vtile")
    nc.sync.dma_start(out=vtile[:], in_=io["vtile"][:, :])
    labels = cpool.tile([128, NB], F32, tag="labels")
    nc.scalar.dma_start(out=labels[:], in_=io["labels"][:, :])
    identb = cpool.tile([128, 128], BF16, tag="identb")
    nc.scalar.dma_start(out=identb[:], in_=io["identb"][:, :])
    wrt = [cpool.tile([128, NPR], BF16, tag=f"wrt{j}", name=f"wrt{j}")
           for j in range(3)]
    wrg = [cpool.tile([128, NPR], BF16, tag=f"wrg{j}", name=f"wrg{j}")
           for j in range(3)]
    for j in range(3):
        nc.sync.dma_start(out=wrt[j][:], in_=io["wrt"][128 * j:128 * (j + 1), :])
    for j in range(3):
        nc.scalar.dma_start(out=wrg[j][:], in_=io["wrg"][128 * j:128 * (j + 1), :])


    ones1 = cpool.tile([1, H], BF16, tag="ones1")
    nc.vector.memset(ones1[:], 1.0)
    ones128 = cpool.tile([128, 1], F32, tag="ones128")
    nc.vector.memset(ones128[:], 1.0)

    # ---- time MLP -> broadcast coefficient tiles ctile[c] --------------
    tvh_bufs = [cpool.tile([MH, 128], BF16, tag=f"tvhb{i}", name=f"tvhb{i}")
                for i in range(3)]
    for i in range(3):
        nc.vector.memset(tvh_bufs[i][:], 1.0)
    ctiles = []
    for c in range(NB):
        bcast = ptv.tile([H, 128], F32, tag="ptv", name=f"bcast{c}")
        nc.tensor.matmul(bcast[:], ones1[:], times[0:1, 128 * c:128 * (c + 1)],
                         start=True, stop=True)
        h1T = tvpool.tile([H, 128], BF16, tag="h1T")
        nc.scalar.activation(h1T[:], bcast[:], AF.Tanh, bias=h1b[:],
                             scale=h1k[:])
        tvp = ptv.tile([H, 128], F32, tag="ptv", name=f"tvp{c}")
        nc.tensor.matmul(tvp[:], h2k[:], h1T[:], start=True, stop=True)
        tvhT = tvh_bufs[c % 3]
        nc.scalar.activation(tvhT[0:H, :], tvp[:], AF.Tanh, bias=h2b[:])
        cwp = pmm.tile([128, NPR], F32, tag="mp", name=f"cwp{c}")
        nc.tensor.matmul(cwp[:], tvhT[:], vtile[:], start=True, stop=True)
        ct = cpool.tile([128, NPR], BF16, tag=f"ct{c}", name=f"ct{c}")
        nc.vector.tensor_copy(ct[:], cwp[:])
        ctiles.append(ct)

    # ---- per-chunk branch contraction ---------------------------------
    et12 = {}

    def group_transpose(br, g, emb):
        t = wpool.tile([128, 12, 128], BF16, tag=f"et12_{br}",
                       name=f"et12_{br}{g}")
        nc.sync.dma_start_transpose(
            t[:], emb[g][:].rearrange("p c e -> p (c e)"))
        et12[br, g] = t

    def branch_mv(br, c, wr, emb, mv_out):
        if USE_DMA_TRANSPOSE:
            if (br, c // 4) not in et12:
                group_transpose(br, c // 4, emb)
            t = et12[br, c // 4]
            lhs = [t[:, (c % 4) * 3 + j, :] for j in range(3)]
        else:
            lhs = []
            for j in range(3):
                tpp = ptr.tile([128, 128], BF16, tag="pt", name=f"pt{br}{c}{j}")
                nc.tensor.transpose(
                    tpp[:], emb[c // 4][:, c % 4, 128 * j:128 * (j + 1)],
                    identb[:])
                et = wpool.tile([128, 128], BF16, tag=f"et{j}_{br}",
                                name=f"et{j}_{br}{c}")
                nc.vector.tensor_copy(et[:], tpp[:])
                lhs.append(et[:])
        mp = pmm.tile([128, NPR], F32, tag="mp", name=f"mp_{br}{c}")
        for j in range(3):
            nc.tensor.matmul(mp[:], lhs[j], wr[j][:], start=(j == 0),
                             stop=(j == 2))
        ms = wpool.tile([128, NPR], BF16, tag=f"ms_{br}", name=f"ms_{br}{c}")
        nc.scalar.copy(ms[:], mp[:])
        prod = wpool.tile([128, NPR], BF16, tag=f"prod_{br}",
                          name=f"prod_{br}{c}")
        nc.vector.tensor_mul(prod[:], ms[:], ctiles[c][:])
        nc.vector.reduce_sum(
            out=mv_out,
            in_=prod[:].rearrange("p (a k) -> p a k", k=R),
            axis=AX.X,
        )

    mvt = [cpool.tile([128, MH], F32, tag=f"mvt{c}", name=f"mvt{c}")
           for c in range(NB)]
    logits = cpool.tile([128, NB], F32, tag="logits")

    def do_c(c):
        mvc = wpool.tile([128, MH], F32, tag="mvc", name=f"mvc{c}")
        branch_mv("c", c, wrg, emb_c, mvc[:])
        junk = lpool.tile([128, MH], F32, tag="junk", name=f"junk{c}")
        nc.vector.tensor_mul(junk[:], mvt[c][:], mvc[:])
        nc.vector.reduce_sum(out=logits[:, c:c + 1], in_=junk[:], axis=AX.X)

    # per 4-chunk group (matching the gather pieces): t chunks, then c
    for g in range(4):
        for c in range(4 * g, 4 * g + 4):
            branch_mv("t", c, wrt, emb_t, mvt[c][:])
        for c in range(4 * g, 4 * g + 4):
            do_c(c)

    # ---- batched loss tail: softplus(l) - l*y over [128, NB] -----------
    ab = lpool.tile([128, NB], F32, tag="ab")
    nc.scalar.activation(ab[:], logits[:], AF.Abs)
    ex = lpool.tile([128, NB], F32, tag="ex")
    nc.scalar.activation(ex[:], ab[:], AF.Exp, scale=-1.0)
    l1p = lpool.tile([128, NB], F32, tag="l1p")
    nc.scalar.activation(l1p[:], ex[:], AF.Ln, bias=1.0)
    rl = lpool.tile([128, NB], F32, tag="rl")
    nc.scalar.activation(rl[:], logits[:], AF.Relu)
    sp = lpool.tile([128, NB], F32, tag="sp")
    nc.vector.tensor_add(sp[:], rl[:], l1p[:])
    ll = lpool.tile([128, NB], F32, tag="ll")
    nc.vector.tensor_mul(ll[:], logits[:], labels[:])
    dvec = lpool.tile([128, NB], F32, tag="dvec")
    nc.vector.tensor_sub(dvec[:], sp[:], ll[:])

    srow = cpool.tile([128, 1], F32, tag="srow")
    nc.vector.reduce_sum(out=srow[:], in_=dvec[:], axis=AX.X)
    fin = ptv.tile([1, 1], F32, tag="ptv", name="pfin")
    nc.tensor.matmul(fin[:], srow[:], ones128[:], start=True, stop=True)
    res = cpool.tile([1, 1], F32, tag="res")
    nc.scalar.copy(res[:], fin[:])
    nc.sync.dma_start(out=io["out"][:, :], in_=res[:])


_PROGRAM = None


def _get_program():
    global _PROGRAM
    if _PROGRAM is not None:
        return _PROGRAM
    nc = bacc.Bacc("TRN2", target_bir_lowering=False, debug=False,
                   num_devices=N_CORES, num_swdge_queues=4,
                   dynamic_dma_scratch_size=65536)
    io = {
        "ttab": nc.dram_tensor("ttab", [W, EPAD], BF16, kind="ExternalInput").ap(),
        "wrt": nc.dram_tensor("wrt", [EPAD, NPR], BF16, kind="ExternalInput").ap(),
        "wrg": nc.dram_tensor("wrg", [EPAD, NPR], BF16, kind="ExternalInput").ap(),
        "vtile": nc.dram_tensor("vtile", [MH, NPR], BF16, kind="ExternalInput").ap(),
        "h2k": nc.dram_tensor("h2k", [H, H], BF16, kind="ExternalInput").ap(),
        "h2b": nc.dram_tensor("h2b", [H, 1], F32, kind="ExternalInput").ap(),
        "h1k": nc.dram_tensor("h1k", [H, 1], F32, kind="ExternalInput").ap(),
        "h1b": nc.dram_tensor("h1b", [H, 1], F32, kind="ExternalInput").ap(),
        "identb": nc.dram_tensor("identb", [128, 128], BF16, kind="ExternalInput").ap(),
        "times": nc.dram_tensor("times", [1, BC], BF16, kind="ExternalInput").ap(),
        "labels": nc.dram_tensor("labels", [128, NB], F32, kind="ExternalInput").ap(),
        "idx_t": nc.dram_tensor("idx_t", [128, BC // 16], I16, kind="ExternalInput").ap(),
        "idx_c": nc.dram_tensor("idx_c", [128, BC // 16], I16, kind="ExternalInput").ap(),
        "out": nc.dram_tensor("out", [1, 1], F32, kind="ExternalOutput").ap(),
    }
    for s in range(4):
        io[f"ctab{s}"] = nc.dram_tensor(f"ctab{s}", [W, EPAD], BF16,
                                        kind="ExternalInput").ap()
    with tile.TileContext(nc) as tc:
        with ExitStack() as ctx:
            _build_kernel(ctx, tc, io)
    nc.compile()
    _PROGRAM = nc
    return nc


def _pad_table(tab):
    out = np.zeros((V, EPAD), dtype=ml_dtypes.bfloat16)
    out[:, :EMB] = np.asarray(tab).astype(ml_dtypes.bfloat16)
    out[:, EMB] = 1.0
    return out


def _precompute_weights(h1_k, h1_b, h2_k, h2_b, evoke_k, evoke_b,
                        last_k, last_b):
    """Affine rank-R tv basis + folded contraction weights (float64)."""
    h1_k = np.asarray(h1_k, np.float64)
    h1_b = np.asarray(h1_b, np.float64)
    h2_k = np.asarray(h2_k, np.float64)
    h2_b = np.asarray(h2_b, np.float64)
    g = np.linspace(0.0, 1.0, 8193, dtype=np.float64).reshape(-1, 1)
    h1g = np.tanh(g @ h1_k.reshape(1, H) + h1_b.reshape(H))
    tvg = np.tanh(h1g @ h2_k + h2_b.reshape(H))
    m = tvg.mean(0)
    _, _, vt = np.linalg.svd(tvg - m, full_matrices=False)
    v3 = vt[:R - 1].T                                   # [100, R-1]
    b_aff = np.concatenate([m.reshape(-1, 1), v3], 1)   # [100, R]
    vaff_h = np.zeros((MH, R))
    vaff_h[:H, 1:] = v3
    vaff_h[H, 0] = 1.0
    vaff_h[H, 1:] = -(m @ v3)

    evoke_pad = np.zeros((EPAD, H * H))
    evoke_pad[:EMB] = np.asarray(evoke_k, np.float64)
    evoke_pad[EMB] = np.asarray(evoke_b, np.float64)
    wr = (evoke_pad.reshape(EPAD * H, H) @ b_aff).reshape(EPAD, H, R)
    wr_full = np.zeros((EPAD, MH, R))
    wr_full[:, :H, :] = wr
    wr_full[EMB, H, 0] = 1.0
    lastkh = np.vstack([np.asarray(last_k, np.float64),
                        np.asarray(last_b, np.float64).reshape(1, EMB)])
    gh = lastkh @ lastkh.T
    wrgh = np.einsum('epk,pq->eqk', wr_full, gh)

    wrt = wr_full.reshape(EPAD, NPR).astype(ml_dtypes.bfloat16)
    wrg = wrgh.reshape(EPAD, NPR).astype(ml_dtypes.bfloat16)
    vtile = np.tile(vaff_h, (1, MH)).astype(ml_dtypes.bfloat16)
    h2kc = h2_k.astype(ml_dtypes.bfloat16)
    h2bc = h2_b.reshape(H, 1).astype(np.float32).copy()
    h1kc = h1_k.reshape(1, H).T.astype(np.float32).copy()
    h1bc = h1_b.reshape(H, 1).astype(np.float32).copy()
    return wrt, wrg, vtile, h2kc, h2bc, h1kc, h1bc


def build_in_maps(targets, contexts, times, labels, targetemb, contextemb,
                  h1_k, h1_b, h2_k, h2_b, evoke_k, evoke_b, last_k, last_b):
    global LAST_PERMS
    ttab = _pad_table(targetemb)
    ctab = _pad_table(contextemb)
    wrt, wrg, vtile, h2kc, h2bc, h1kc, h1bc = _precompute_weights(
        h1_k, h1_b, h2_k, h2_b, evoke_k, evoke_b, last_k, last_b)
    identb = np.eye(128, dtype=ml_dtypes.bfloat16)
    targets = np.asarray(targets).astype(np.int64)
    contexts = np.asarray(contexts).astype(np.int64)
    times = np.asarray(times).astype(np.float32)
    labels = np.asarray(labels).astype(np.float32)

    order_t = np.argsort(targets, kind="stable")
    in_maps = []
    perms = []
    for k in range(N_CORES):
        i_k = order_t[k * BC:(k + 1) * BC]
        j_k = i_k[np.argsort(contexts[i_k], kind="stable")]
        perms.append(j_k)
        tv = targets[j_k]
        cv = contexts[j_k]
        off_t = min(int(tv.min()), V - W)
        t_loc = tv - off_t
        assert t_loc.min() >= 0 and t_loc.max() < W, "t window overflow"
        m = {
            "ttab": ttab[off_t:off_t + W],
            "wrt": wrt, "wrg": wrg, "vtile": vtile,
            "h2k": h2kc, "h2b": h2bc,
            "h1k": h1kc, "h1b": h1bc, "identb": identb,
            "times": times[j_k].astype(ml_dtypes.bfloat16).reshape(1, BC),
            "labels": labels[j_k].reshape(NB, 128).T.copy(),
            "idx_t": _wrap16(t_loc),
        }
        c_loc = np.empty(BC, dtype=np.int64)
        for s in range(4):
            seg = cv[SEG * s:SEG * (s + 1)]
            base = min(int(seg[0]), V - W)
            loc = seg - base
            assert loc.min() >= 0 and loc.max() < W, "c window overflow"
            c_loc[SEG * s:SEG * (s + 1)] = loc
            m[f"ctab{s}"] = ctab[base:base + W]
        m["idx_c"] = _wrap16(c_loc)
        in_maps.append(m)
    LAST_PERMS = perms
    return in_maps


def kernel(**inputs) -> np.ndarray:
    nc = _get_program()
    in_maps = build_in_maps(**inputs)
    r = run_bass_kernel_spmd(nc, in_maps, list(range(N_CORES)))
    total = np.float64(0.0)
    for m in r.results:
        total += np.float64(m["out"][0, 0])
    return np.float32(total / B)
